# revision 12
# baseline (speedup 1.0000x reference)
"""GAT (2-layer, PyG-style) on 8 Trainium2 NeuronCores via Bass/Tile.

Strategy (dst-sharded, degree-sorted tiles), v2:
- Nodes sharded by dst across 8 cores (12500 each). Per core, dsts are
  degree-sorted and grouped into 98 tiles of 128 (partition = dst).
- Per tile, column 0 gathers the dst's own table row (serving both the
  self-loop edge and al_dst); remaining columns hold in-edges; invalid
  slots gather a zeroed sentinel row so no masking stream is needed.
- Layer tables are 4-node-packed rows (<=32767 rows, int16 dma_gather
  indices); the 4-way select runs as int32-pair predicated copies (half
  the DVE elements of a bf16 select).
- exp(LeakyReLU(a+b)) == max(exp(a)exp(b), exp(a/5)exp(b/5)^...) trick:
  tables store E1=exp(al_src), E2=exp(0.2*al_src); per-tile D1=exp(al_dst),
  D2=exp(0.2*al_dst); per-edge weight w = max(E1*D1, E2*D2). No per-edge
  exp and no LeakyReLU on the hot path.
- h stored c-major so the alpha*h multiply runs in the DVE 2x packed mode
  with the weight broadcast on a middle axis.
- Node phase processes 8 x-tiles per DMA; host pre-permutes x columns so
  each slab writes table1 with a single strided DMA.
- Per-tile accumulators land in a persistent SBUF buffer; softmax
  normalization, ELU and the layer-2 projection run as batched ops after
  the edge phase (no per-tile serial finalize chain).
- Layer-2 per-node features are exchanged via an on-chip AllGather of
  4-packed shards in core-local permuted order.
"""

import numpy as np
import ml_dtypes

BF16 = ml_dtypes.bfloat16

N = 100_000
E = 3_200_000
IN = 128
H1, C1 = 8, 8
HID = H1 * C1          # 64
OUT = 2
NEG = 0.2
NCORES = 8
ND = N // NCORES       # dsts per core: 12500
NT = 98                # tiles per core (98*128 = 12544)
PT = NT * 128          # padded dst slots per core
NPAD = 100_352         # x padded to 98*1024
T1ROWS = NPAD // 4     # 25088 4-pack rows
REC1 = 96              # elems per node record: h(64)|E1(8)|E2(8)|aldst(8)|pad
T1W = 4 * REC1         # 384 bf16 per table1 row (768B)
SENT_ROW = 25_000      # row holding node 100000 (zeroed sentinel)
T2SROWS = PT // 4 + 1  # per-core layer2 shard rows (+1 zero sentinel row)
T2ROWS = T2SROWS * NCORES     # 25096
REC2 = 32              # z0 z1 E1 E2 aldst 0...
T2W = 4 * REC2         # 128 bf16 per table2 row (256B)
PARTC = 32             # columns per tile-part
CPC = 8                # columns per dma_gather call (65 descs; ring holds one
                       # in-flight call per queue -- larger calls wedge)
XS = 8                 # x-tiles per node-phase slab


def _wrap_idx(flat):
    """int16 index array -> [128, n/16] wrapped-in-16-partitions, replicated x8."""
    n = flat.shape[0]
    assert n % 16 == 0
    w = flat.reshape(n // 16, 16).T            # [16, n/16]
    return np.tile(w, (8, 1)).astype(np.int16)  # [128, n/16]


def _plan(src, dst):
    """Host-side index planning. Returns per-core data arrays + common schedule."""
    core = dst // ND
    dloc = dst % ND

    per_core = []
    for c in range(NCORES):
        m = core == c
        s_c = src[m]
        d_c = dloc[m]
        deg = np.bincount(d_c, minlength=ND)  # in-edges, no self loop yet
        order = np.argsort(-deg, kind="stable")  # degree-desc permutation
        perm = np.full(PT, -1, dtype=np.int64)
        perm[:ND] = order
        degp = np.zeros(PT, dtype=np.int64)
        degp[:ND] = deg[order]
        # group in-edges by dst for slot filling
        sort_by_d = np.argsort(d_c, kind="stable")
        s_sorted = s_c[sort_by_d]
        starts = np.zeros(ND + 1, dtype=np.int64)
        np.cumsum(deg, out=starts[1:])
        per_core.append(dict(perm=perm, degp=degp, s_sorted=s_sorted, starts=starts))

    # common K_t schedule: columns per tile = 1 (self/dst col) + max in-degree
    K = np.zeros(NT, dtype=np.int64)
    for t in range(NT):
        mx = 0
        for c in range(NCORES):
            d = per_core[c]["degp"][t * 128 : (t + 1) * 128]
            mx = max(mx, int(d.max()) if d.size else 0)
        K[t] = mx + 1
    col0 = np.zeros(NT, dtype=np.int64)
    pos = 0
    for t in range(NT):
        col0[t] = pos
        pos += K[t]
    ncols = pos

    # per-core slot arrays: node id per (col, partition); -1 = invalid
    datas = []
    for c in range(NCORES):
        pc = per_core[c]
        perm, degp, s_sorted, starts = (
            pc["perm"], pc["degp"], pc["s_sorted"], pc["starts"],
        )
        node1 = np.full((ncols, 128), -1, dtype=np.int64)
        for t in range(NT):
            base = col0[t]
            d_orig = perm[t * 128 : (t + 1) * 128]           # local dst ids, -1 pad
            real = d_orig >= 0
            dg = np.where(real, d_orig, 0)
            node1[base, real] = (c * ND + dg)[real]
            kt = int(K[t])
            if kt > 1:
                st = starts[dg]
                cnt = degp[t * 128 : (t + 1) * 128]
                for j in range(1, kt):
                    sel = (j - 1 < cnt) & real
                    idxs = st + (j - 1)
                    node1[base + j, sel] = s_sorted[np.where(sel, idxs, 0)][sel]
        datas.append(dict(node1=node1, perm=pc["perm"]))
    return datas, K, col0, ncols


def _pack_inputs(datas, gpos_of_node, parts, sent_pos):
    """Build per-core device input arrays from the slot plan."""
    per_core_inputs = []
    for c in range(NCORES):
        node1 = datas[c]["node1"]      # [ncols, 128], -1 invalid
        valid = node1 >= 0
        n1 = np.where(valid, node1, N)              # sentinel node 100000
        idx1_flat = (n1 // 4).astype(np.int16)
        oh1 = np.eye(4, dtype=np.int8)[n1 % 4]      # [ncols, 128, 4]

        g = np.where(valid, gpos_of_node[np.where(valid, node1, 0)],
                     sent_pos[c])
        idx2_flat = (g // 4).astype(np.int16)
        oh2 = np.eye(4, dtype=np.int8)[g % 4]

        w1l, w2l = [], []
        for (c0, nc_) in parts:
            w1l.append(_wrap_idx(idx1_flat[c0 : c0 + nc_].reshape(-1)))
            w2l.append(_wrap_idx(idx2_flat[c0 : c0 + nc_].reshape(-1)))
        per_core_inputs.append(dict(
            idx1=np.concatenate(w1l, axis=1),
            idx2=np.concatenate(w2l, axis=1),
            oh1=np.ascontiguousarray(oh1.transpose(1, 0, 2)),
            oh2=np.ascontiguousarray(oh2.transpose(1, 0, 2)),
        ))
    return per_core_inputs


_BUILD_CACHE = {}


def _build(K, col0, totc):
    import concourse.bass as bass
    import concourse.bacc as bacc
    import concourse.mybir as mybir
    import concourse.tile as tile
    from concourse.masks import make_identity

    f32 = mybir.dt.float32
    bf16 = mybir.dt.bfloat16
    i16 = mybir.dt.int16
    i32 = mybir.dt.int32
    i8 = mybir.dt.int8
    AX = mybir.AxisListType.X
    OP = mybir.AluOpType
    ACT = mybir.ActivationFunctionType

    ncols = int(K.sum())

    nc = bacc.Bacc("TRN2", target_bir_lowering=False, debug=False,
                   num_devices=NCORES, num_swdge_queues=4)

    x = nc.dram_tensor("x", [IN, NPAD], bf16, kind="ExternalInput")
    w1e = nc.dram_tensor("w1e", [IN, 80], bf16, kind="ExternalInput")
    w2e = nc.dram_tensor("w2e", [HID, 4], bf16, kind="ExternalInput")
    idx1 = nc.dram_tensor("idx1", [128, totc], i16, kind="ExternalInput")
    idx2 = nc.dram_tensor("idx2", [128, totc], i16, kind="ExternalInput")
    oh1 = nc.dram_tensor("oh1", [128, ncols, 4], i8, kind="ExternalInput")
    oh2 = nc.dram_tensor("oh2", [128, ncols, 4], i8, kind="ExternalInput")
    b1e = nc.dram_tensor("b1e", [128, HID], f32, kind="ExternalInput")
    b2e = nc.dram_tensor("b2e", [128, OUT], f32, kind="ExternalInput")

    table1 = nc.dram_tensor("table1", [T1ROWS, T1W], bf16, kind="Internal")
    t2shard = nc.dram_tensor("t2shard", [T2SROWS, T2W], bf16, kind="Internal")
    table2 = nc.dram_tensor("table2", [T2ROWS, T2W], bf16, kind="Internal",
                            addr_space="Shared")
    outp = nc.dram_tensor("outp", [PT, OUT], f32, kind="ExternalOutput")

    with tile.TileContext(nc) as tc:
        with (
            tc.tile_pool(name="const", bufs=1) as cpool,
            tc.tile_pool(name="node", bufs=3) as npool,
            tc.tile_pool(name="npsum", bufs=4, space="PSUM") as npsum,
            tc.tile_pool(name="gth", bufs=2) as gpool,
            tc.tile_pool(name="edge", bufs=3) as epool,
            tc.tile_pool(name="esmall", bufs=4) as espool,
            tc.tile_pool(name="stream", bufs=6) as spool,
            tc.tile_pool(name="accs", bufs=2) as apool,
            tc.tile_pool(name="fin", bufs=2) as fpool,
            tc.tile_pool(name="fpsum", bufs=2, space="PSUM") as fpsum,
        ):
            ident = cpool.tile([128, 128], bf16)
            make_identity(nc, ident[:])
            w1es = cpool.tile([IN, 80], bf16)
            nc.sync.dma_start(out=w1es[:], in_=w1e[:])
            w2es = cpool.tile([HID, 4], bf16)
            nc.sync.dma_start(out=w2es[:], in_=w2e[:])
            b1es = cpool.tile([128, HID], f32)
            nc.sync.dma_start(out=b1es[:], in_=b1e[:])
            b2es = cpool.tile([128, OUT], f32)
            nc.sync.dma_start(out=b2es[:], in_=b2e[:])
            # persistent accumulators + staging
            accT = cpool.tile([128, NT, 72], f32)       # num(64 cmajor)|den(8)
            accT2 = cpool.tile([128, NT, 3], f32)       # num(2)|den(1)
            zball = cpool.tile([128, NT, HID], bf16)    # elu output, c-major
            t2s = cpool.tile([128, NT, REC2], bf16)     # layer2 table staging
            t2raw = cpool.tile([128, NT, 4], f32)       # raw g2 results
            zrow = cpool.tile([128, T1W], bf16)         # sentinel zero row
            nc.vector.memset(zrow[:], 0.0)
            nc.vector.memset(t2s[:], 0.0)

            # ---- node phase: table1 records per node, 8 x-tiles per slab.
            # Host permutes x columns: slab col (i*128+p) holds node
            # sl*1024 + 8p + i, so the table write is a single strided DMA.
            for sl in range(NPAD // 1024):
                eng = nc.sync
                xs = npool.tile([IN, 1024], bf16, tag="xs")
                eng.dma_start(out=xs[:], in_=x[:, sl * 1024 : (sl + 1) * 1024])
                t1s = npool.tile([128, XS, REC1], bf16, tag="t1s")
                for hf in range(2):
                    g1 = npsum.tile([128, 4, 80], f32, tag="g1")
                    for i in range(4):
                        ti = hf * 4 + i
                        nc.tensor.matmul(
                            out=g1[:, i, :],
                            lhsT=xs[:, ti * 128 : (ti + 1) * 128],
                            rhs=w1es[:], start=True, stop=True)
                    sel = t1s[:, hf * 4 : hf * 4 + 4, :]
                    # h (already c-major from the W1 column permutation) + b1
                    nc.vector.tensor_tensor(
                        out=sel[:, :, 0:HID], in0=g1[:, :, 0:HID],
                        in1=b1es[:].unsqueeze(1).to_broadcast([128, 4, HID]),
                        op=OP.add)
                    nc.scalar.activation(out=sel[:, :, 64:72],
                                         in_=g1[:, :, 64:72], func=ACT.Exp)
                    nc.scalar.activation(out=sel[:, :, 72:80],
                                         in_=g1[:, :, 64:72], func=ACT.Exp,
                                         scale=NEG)
                    nc.vector.tensor_copy(out=sel[:, :, 80:88],
                                          in_=g1[:, :, 72:80])
                    nc.vector.memset(sel[:, :, 88:96], 0.0)
                # one strided DMA writes 1024 node records
                dst = table1[:].rearrange("r w -> (r w)").rearrange(
                    "(s p i1 i0 v) -> s p i1 i0 v",
                    p=128, i1=2, i0=4, v=REC1)[sl]
                eng.dma_start(out=dst, in_=t1s[:])
            # zero the sentinel row (node 100000)
            nc.sync.dma_start(out=table1[SENT_ROW : SENT_ROW + 1, :],
                              in_=zrow[0:1, :])

            # ---- edge phase runner
            def edge_phase(layer):
                if layer == 1:
                    idxT, ohT, tabT, EW, RC = idx1, oh1, table1, T1W, REC1
                    NH, NV = H1, HID
                    acc = accT
                else:
                    idxT, ohT, tabT, EW, RC = idx2, oh2, table2, T2W, REC2
                    NH, NV = 1, OUT
                    acc = accT2
                RCI = RC // 2           # record in int32 elems
                SEL = (NV + 2 * NH) // 2  # int32 width of contiguous select
                # flat part list with stream lookahead: idx on sync,
                # oh on scalar, dispatched LOOK parts ahead of compute
                plist = []
                ioff = 0
                for t in range(NT):
                    kt_full = int(K[t])
                    for p0 in range(0, kt_full, PARTC):
                        kt = min(PARTC, kt_full - p0)
                        plist.append((t, p0, kt, ioff, int(col0[t]) + p0))
                        ioff += kt * 8

                stream_tiles = {}

                def dispatch_streams(i):
                    _, _, kt, ioff_, col = plist[i]
                    idx_t = spool.tile([128, PARTC * 8], i16,
                                       tag=f"ix{layer}")
                    nc.sync.dma_start(out=idx_t[:, 0 : kt * 8],
                                      in_=idxT[:, ioff_ : ioff_ + kt * 8])
                    oh_t = spool.tile([128, PARTC, 4], i8, tag=f"oh{layer}")
                    nc.scalar.dma_start(out=oh_t[:, 0:kt, :],
                                        in_=ohT[:, col : col + kt, :])
                    stream_tiles[i] = (idx_t, oh_t)

                LOOK = 3
                for i in range(min(LOOK, len(plist))):
                    dispatch_streams(i)
                gq = 0
                D1 = D2 = None
                for i, (t, p0, kt, ioff_, col) in enumerate(plist):
                    if True:
                        if i + LOOK < len(plist):
                            dispatch_streams(i + LOOK)
                        first = p0 == 0
                        idx_t, oh_t = stream_tiles.pop(i)
                        gt = gpool.tile([128, PARTC, EW], bf16,
                                        tag=f"gt{layer}")
                        for cc in range(0, kt, CPC):
                            ncc = min(CPC, kt - cc)
                            nc.gpsimd.dma_gather(
                                gt[:, cc : cc + ncc, :], tabT[:],
                                idx_t[:, cc * 8 : (cc + ncc) * 8],
                                ncc * 128, ncc * 128, EW, queue_num=gq % 4)
                            gq += 1

                        gti = gt[:, 0:kt, :].bitcast(i32)   # [128, kt, EW/2]
                        V = epool.tile([128, PARTC, NV + 2 * NH], bf16,
                                       tag=f"V{layer}")
                        Vi = V[:, 0:kt, :].bitcast(i32)
                        for s in range(4):
                            nc.vector.copy_predicated(
                                out=Vi,
                                mask=oh_t[:, 0:kt, s : s + 1].to_broadcast(
                                    [128, kt, SEL]),
                                data=gti[:, :, s * RCI : s * RCI + SEL])
                        if first:
                            adt = espool.tile([128, 1, 2 * ((NH + 1) // 2)],
                                             bf16, tag=f"adt{layer}")
                            adti = adt[:].bitcast(i32)
                            na = adti.shape[-1]
                            for s in range(4):
                                nc.vector.copy_predicated(
                                    out=adti,
                                    mask=oh_t[:, 0:1, s : s + 1].to_broadcast(
                                        [128, 1, na]),
                                    data=gti[:, 0:1,
                                             s * RCI + SEL : s * RCI + SEL + na])
                            D1 = espool.tile([128, 1, NH], bf16,
                                            tag=f"D1{layer}")
                            D2 = espool.tile([128, 1, NH], bf16,
                                            tag=f"D2{layer}")
                            nc.scalar.activation(out=D1[:], in_=adt[:, :, 0:NH],
                                                 func=ACT.Exp)
                            nc.scalar.activation(out=D2[:], in_=adt[:, :, 0:NH],
                                                 func=ACT.Exp, scale=NEG)
                        m1 = espool.tile([128, PARTC, NH], bf16,
                                        tag=f"m1{layer}")
                        m2 = espool.tile([128, PARTC, NH], bf16,
                                        tag=f"m2{layer}")
                        w = espool.tile([128, PARTC, NH], bf16,
                                       tag=f"w{layer}")
                        nc.vector.tensor_tensor(
                            out=m1[:, 0:kt, :], in0=V[:, 0:kt, NV : NV + NH],
                            in1=D1[:].to_broadcast([128, kt, NH]), op=OP.mult)
                        nc.vector.tensor_tensor(
                            out=m2[:, 0:kt, :],
                            in0=V[:, 0:kt, NV + NH : NV + 2 * NH],
                            in1=D2[:].to_broadcast([128, kt, NH]), op=OP.mult)
                        nc.vector.tensor_tensor(
                            out=w[:, 0:kt, :], in0=m1[:, 0:kt, :],
                            in1=m2[:, 0:kt, :], op=OP.max)
                        Wj = epool.tile([128, PARTC, NV], bf16,
                                        tag=f"Wj{layer}")
                        if layer == 1:
                            # c-major V, weight broadcast on middle axis -> 2x
                            nc.vector.tensor_tensor(
                                out=Wj[:, 0:kt, :].rearrange(
                                    "p j (c h) -> p j c h", h=H1),
                                in0=V[:, 0:kt, 0:NV].rearrange(
                                    "p j (c h) -> p j c h", h=H1),
                                in1=w[:, 0:kt, :].unsqueeze(2).to_broadcast(
                                    [128, kt, C1, H1]),
                                op=OP.mult)
                        else:
                            nc.vector.tensor_tensor(
                                out=Wj[:, 0:kt, :], in0=V[:, 0:kt, 0:NV],
                                in1=w[:, 0:kt, :].to_broadcast([128, kt, NV]),
                                op=OP.mult)
                        if first:
                            nc.vector.tensor_reduce(
                                out=acc[:, t, 0:NV],
                                in_=Wj[:, 0:kt, :].rearrange("p j f -> p f j"),
                                axis=AX, op=OP.add)
                            nc.vector.tensor_reduce(
                                out=acc[:, t, NV : NV + NH],
                                in_=w[:, 0:kt, :].rearrange("p j h -> p h j"),
                                axis=AX, op=OP.add)
                        else:
                            rn = apool.tile([128, NV + NH], f32,
                                            tag=f"rn{layer}")
                            nc.vector.tensor_reduce(
                                out=rn[:, 0:NV],
                                in_=Wj[:, 0:kt, :].rearrange("p j f -> p f j"),
                                axis=AX, op=OP.add)
                            nc.vector.tensor_reduce(
                                out=rn[:, NV : NV + NH],
                                in_=w[:, 0:kt, :].rearrange("p j h -> p h j"),
                                axis=AX, op=OP.add)
                            nc.vector.tensor_tensor(
                                out=acc[:, t, 0 : NV + NH],
                                in0=acc[:, t, 0 : NV + NH], in1=rn[:],
                                op=OP.add)

            edge_phase(1)

            # ---- batched layer-1 finalize: softmax divide + ELU, 14-tile chunks
            CH = 7
            for ch in range(NT // CH):
                sl = accT[:, ch * CH : (ch + 1) * CH, :]
                den = fpool.tile([128, CH, H1], f32, tag="den")
                nc.vector.tensor_scalar(out=den[:], in0=sl[:, :, 64:72],
                                        scalar1=1e-12, scalar2=None,
                                        op0=OP.max)
                rd = fpool.tile([128, CH, H1], f32, tag="rd")
                nc.vector.reciprocal(out=rd[:], in_=den[:])
                z = fpool.tile([128, CH, HID], f32, tag="z")
                nc.vector.tensor_tensor(
                    out=z[:].rearrange("p t (c h) -> p t c h", h=H1),
                    in0=sl[:, :, 0:HID].rearrange("p t (c h) -> p t c h", h=H1),
                    in1=rd[:].unsqueeze(2).to_broadcast([128, CH, C1, H1]),
                    op=OP.mult)
                # elu
                zp = fpool.tile([128, CH, HID], f32, tag="zp")
                nc.vector.tensor_scalar(out=zp[:], in0=z[:], scalar1=0.0,
                                        scalar2=None, op0=OP.max)
                nc.vector.tensor_scalar(out=z[:], in0=z[:], scalar1=0.0,
                                        scalar2=None, op0=OP.min)
                ez = fpool.tile([128, CH, HID], f32, tag="ez")
                nc.scalar.activation(out=ez[:], in_=z[:], func=ACT.Exp)
                nc.vector.tensor_scalar(out=ez[:], in0=ez[:], scalar1=-1.0,
                                        scalar2=None, op0=OP.add)
                nc.vector.tensor_tensor(
                    out=zball[:, ch * CH : (ch + 1) * CH, :], in0=zp[:],
                    in1=ez[:], op=OP.add)

            # ---- layer-2 projection per tile + staging
            for t in range(NT):
                zT_ps = fpsum.tile([HID, 128], bf16, tag="zTp")
                nc.tensor.transpose(out=zT_ps[:], in_=zball[:, t, :],
                                    identity=ident[:])
                zTs = fpool.tile([HID, 128], bf16, tag="zTs")
                nc.vector.tensor_copy(out=zTs[:], in_=zT_ps[:])
                g2 = fpsum.tile([128, 4], f32, tag="g2p")
                nc.tensor.matmul(out=g2[:], lhsT=zTs[:], rhs=w2es[:],
                                 start=True, stop=True)
                nc.vector.tensor_copy(out=t2raw[:, t, :], in_=g2[:])
            # batched staging ops
            nc.vector.tensor_tensor(
                out=t2s[:, :, 0:OUT], in0=t2raw[:, :, 0:OUT],
                in1=b2es[:].unsqueeze(1).to_broadcast([128, NT, OUT]),
                op=OP.add)
            nc.scalar.activation(out=t2s[:, :, 2:3], in_=t2raw[:, :, 2:3],
                                 func=ACT.Exp)
            nc.scalar.activation(out=t2s[:, :, 3:4], in_=t2raw[:, :, 2:3],
                                 func=ACT.Exp, scale=NEG)
            nc.vector.tensor_copy(out=t2s[:, :, 4:5], in_=t2raw[:, :, 3:4])
            # single strided DMA: slot (p, t) -> position p*98+t
            t2dst = t2shard[0 : PT // 4, :].rearrange("r w -> (r w)").rearrange(
                "(p t v) -> p t v", p=128, t=NT)
            nc.sync.dma_start(out=t2dst, in_=t2s[:])
            # zero sentinel row (gathered by invalid layer-2 slots)
            nc.sync.dma_start(out=t2shard[PT // 4 : T2SROWS, :],
                              in_=zrow[0:1, 0:T2W])

            # ---- exchange layer-2 node features
            nc.gpsimd.collective_compute(
                "AllGather",
                OP.bypass,
                replica_groups=[list(range(NCORES))],
                ins=[t2shard[:]],
                outs=[table2[:]],
            )

            edge_phase(2)

            # ---- batched layer-2 finalize
            den2 = fpool.tile([128, NT, 1], f32, tag="den2")
            nc.vector.tensor_scalar(out=den2[:], in0=accT2[:, :, 2:3],
                                    scalar1=1e-12, scalar2=None, op0=OP.max)
            rd2 = fpool.tile([128, NT, 1], f32, tag="rd2")
            nc.vector.reciprocal(out=rd2[:], in_=den2[:])
            o2 = fpool.tile([128, NT, OUT], f32, tag="o2")
            nc.vector.tensor_tensor(
                out=o2[:], in0=accT2[:, :, 0:OUT],
                in1=rd2[:].to_broadcast([128, NT, OUT]), op=OP.mult)
            odst = outp[:].rearrange("r w -> (r w)").rearrange(
                "(p t v) -> p t v", p=128, t=NT)
            nc.sync.dma_start(out=odst, in_=o2[:])

    nc.compile()
    return nc


def kernel(**inputs):
    from concourse.bass_utils import run_bass_kernel_spmd

    x = np.asarray(inputs["x"], dtype=np.float32)
    ei = np.asarray(inputs["edge_index"]).astype(np.int64)
    w1 = np.asarray(inputs["W1"], dtype=np.float32)
    a1s = np.asarray(inputs["a1_src"], dtype=np.float32)
    a1d = np.asarray(inputs["a1_dst"], dtype=np.float32)
    b1 = np.asarray(inputs["b1"], dtype=np.float32)
    w2 = np.asarray(inputs["W2"], dtype=np.float32)
    a2s = np.asarray(inputs["a2_src"], dtype=np.float32)
    a2d = np.asarray(inputs["a2_dst"], dtype=np.float32)
    b2 = np.asarray(inputs["b2"], dtype=np.float32)

    src = ei[0]
    dst = ei[1]

    datas, K, col0, ncols = _plan(src, dst)
    parts = []
    for t in range(NT):
        for c in range(0, int(K[t]), PARTC):
            parts.append((int(col0[t]) + c, min(PARTC, int(K[t]) - c)))
    totc = sum(nc_ * 8 for _, nc_ in parts)

    # global position of each node for the L2 table: pos = c*4*T2SROWS +
    # p*98 + t where the node is dst slot (t, p) on core c.
    gpos_of_node = np.zeros(N, dtype=np.int64)
    s_old = np.arange(PT)
    tt = s_old // 128
    pp = s_old % 128
    pos_of_slot = pp * NT + tt
    for c in range(NCORES):
        perm = datas[c]["perm"]  # [PT] local dst ids (or -1)
        real = perm >= 0
        gpos_of_node[c * ND + perm[real]] = c * 4 * T2SROWS + pos_of_slot[real]
    sent_pos = np.array([c * 4 * T2SROWS + PT for c in range(NCORES)],
                        dtype=np.int64)

    per_core = _pack_inputs(datas, gpos_of_node, parts, sent_pos)

    # weights: w1e = [W1 | W1@A1s | W1@A1d], h block c-major downstream
    A1s = np.zeros((HID, H1), dtype=np.float32)
    A1d = np.zeros((HID, H1), dtype=np.float32)
    for h in range(H1):
        A1s[h * C1 : (h + 1) * C1, h] = a1s[h]
        A1d[h * C1 : (h + 1) * C1, h] = a1d[h]
    # h block emitted c-major directly: permute W1 columns (and b1)
    cm = (np.arange(HID) % H1) * C1 + (np.arange(HID) // H1)
    w1e = np.concatenate([w1[:, cm], w1 @ A1s, w1 @ A1d], axis=1)   # [128, 80]
    w2cm = w2[cm]
    w2e = np.concatenate([w2cm, w2cm @ a2s.T, w2cm @ a2d.T], axis=1)  # [64, 4]
    b1e = np.tile(b1[cm][None, :], (128, 1)).astype(np.float32)
    b2e = np.tile(b2[None, :], (128, 1)).astype(np.float32)

    # x: pad and permute columns (slab col i*128+p holds node sl*1024+8p+i)
    xp = np.zeros((NPAD, IN), dtype=np.float32)
    xp[:N] = x
    j = np.arange(1024)
    perm1024 = 8 * (j % 128) + j // 128
    permall = (np.arange(NPAD).reshape(-1, 1024) // 1024) * 1024
    permall = (permall + perm1024[None, :]).reshape(-1)
    xpT = np.ascontiguousarray(xp[permall].T.astype(BF16))

    key = (totc, tuple(K.tolist()))
    if key not in _BUILD_CACHE:
        _BUILD_CACHE[key] = _build(K, col0, totc)
    nc = _BUILD_CACHE[key]

    common = dict(x=xpT, w1e=w1e.astype(BF16), w2e=w2e.astype(BF16),
                  b1e=b1e, b2e=b2e)
    in_maps = []
    for c in range(NCORES):
        m = dict(common)
        m.update(per_core[c])
        in_maps.append(m)

    global _LAST_IN_MAPS
    _LAST_IN_MAPS = in_maps
    res = run_bass_kernel_spmd(nc, in_maps, list(range(NCORES)))

    out = np.zeros((N, OUT), dtype=np.float32)
    for c in range(NCORES):
        op = res.results[c]["outp"]       # [PT, 2], row = p*98+t
        perm = datas[c]["perm"]
        real = perm >= 0
        out[c * ND + perm[real]] = op[pos_of_slot[real]]
    return out


# revision 14
# speedup vs baseline: 1.1171x; 1.1171x over previous
"""GAT (2-layer, PyG-style) on 8 Trainium2 NeuronCores via Bass/Tile.

Strategy (dst-sharded, degree-sorted tiles), v2:
- Nodes sharded by dst across 8 cores (12500 each). Per core, dsts are
  degree-sorted and grouped into 98 tiles of 128 (partition = dst).
- Per tile, column 0 gathers the dst's own table row (serving both the
  self-loop edge and al_dst); remaining columns hold in-edges; invalid
  slots gather a zeroed sentinel row so no masking stream is needed.
- Layer tables are 4-node-packed rows (<=32767 rows, int16 dma_gather
  indices); the 4-way select runs as int32-pair predicated copies (half
  the DVE elements of a bf16 select).
- exp(LeakyReLU(a+b)) == max(exp(a)exp(b), exp(a/5)exp(b/5)^...) trick:
  tables store E1=exp(al_src), E2=exp(0.2*al_src); per-tile D1=exp(al_dst),
  D2=exp(0.2*al_dst); per-edge weight w = max(E1*D1, E2*D2). No per-edge
  exp and no LeakyReLU on the hot path.
- h stored c-major so the alpha*h multiply runs in the DVE 2x packed mode
  with the weight broadcast on a middle axis.
- Node phase processes 8 x-tiles per DMA; host pre-permutes x columns so
  each slab writes table1 with a single strided DMA.
- Per-tile accumulators land in a persistent SBUF buffer; softmax
  normalization, ELU and the layer-2 projection run as batched ops after
  the edge phase (no per-tile serial finalize chain).
- Layer-2 per-node features are exchanged via an on-chip AllGather of
  4-packed shards in core-local permuted order.
"""

import numpy as np
import ml_dtypes

BF16 = ml_dtypes.bfloat16

N = 100_000
E = 3_200_000
IN = 128
H1, C1 = 8, 8
HID = H1 * C1          # 64
OUT = 2
NEG = 0.2
NCORES = 8
ND = N // NCORES       # dsts per core: 12500
NT = 98                # tiles per core (98*128 = 12544)
PT = NT * 128          # padded dst slots per core
NPAD = 100_352         # x padded to 98*1024
T1ROWS = NPAD // 4     # 25088 4-pack rows
REC1 = 96              # elems per node record: h(64)|E1(8)|E2(8)|aldst(8)|pad
T1W = 4 * REC1         # 384 bf16 per table1 row (768B)
SENT_ROW = 25_000      # row holding node 100000 (zeroed sentinel)
T2SROWS = PT // 4 + 1  # per-core layer2 shard rows (+1 zero sentinel row)
T2ROWS = T2SROWS * NCORES     # 25096
REC2 = 32              # z0 z1 E1 E2 aldst 0...
T2W = 4 * REC2         # 128 bf16 per table2 row (256B)
PARTC = 32             # columns per tile-part
CPC = 8                # columns per dma_gather call (65 descs; ring holds one
                       # in-flight call per queue -- larger calls wedge)
XS = 8                 # x-tiles per node-phase slab


def _wrap_idx(flat):
    """int16 index array -> [128, n/16] wrapped-in-16-partitions, replicated x8."""
    n = flat.shape[0]
    assert n % 16 == 0
    w = flat.reshape(n // 16, 16).T            # [16, n/16]
    return np.tile(w, (8, 1)).astype(np.int16)  # [128, n/16]


def _plan(src, dst):
    """Host-side index planning. Returns per-core data arrays + common schedule."""
    core = dst // ND
    dloc = dst % ND

    per_core = []
    for c in range(NCORES):
        m = core == c
        s_c = src[m]
        d_c = dloc[m]
        deg = np.bincount(d_c, minlength=ND)  # in-edges, no self loop yet
        order = np.argsort(-deg, kind="stable")  # degree-desc permutation
        perm = np.full(PT, -1, dtype=np.int64)
        perm[:ND] = order
        degp = np.zeros(PT, dtype=np.int64)
        degp[:ND] = deg[order]
        # group in-edges by dst for slot filling
        sort_by_d = np.argsort(d_c, kind="stable")
        s_sorted = s_c[sort_by_d]
        starts = np.zeros(ND + 1, dtype=np.int64)
        np.cumsum(deg, out=starts[1:])
        per_core.append(dict(perm=perm, degp=degp, s_sorted=s_sorted, starts=starts))

    # common K_t schedule: columns per tile = 1 (self/dst col) + max in-degree
    K = np.zeros(NT, dtype=np.int64)
    for t in range(NT):
        mx = 0
        for c in range(NCORES):
            d = per_core[c]["degp"][t * 128 : (t + 1) * 128]
            mx = max(mx, int(d.max()) if d.size else 0)
        K[t] = mx + 1
    col0 = np.zeros(NT, dtype=np.int64)
    pos = 0
    for t in range(NT):
        col0[t] = pos
        pos += K[t]
    ncols = pos

    # per-core slot arrays: node id per (col, partition); -1 = invalid
    datas = []
    for c in range(NCORES):
        pc = per_core[c]
        perm, degp, s_sorted, starts = (
            pc["perm"], pc["degp"], pc["s_sorted"], pc["starts"],
        )
        node1 = np.full((ncols, 128), -1, dtype=np.int64)
        for t in range(NT):
            base = col0[t]
            d_orig = perm[t * 128 : (t + 1) * 128]           # local dst ids, -1 pad
            real = d_orig >= 0
            dg = np.where(real, d_orig, 0)
            node1[base, real] = (c * ND + dg)[real]
            kt = int(K[t])
            if kt > 1:
                st = starts[dg]
                cnt = degp[t * 128 : (t + 1) * 128]
                for j in range(1, kt):
                    sel = (j - 1 < cnt) & real
                    idxs = st + (j - 1)
                    node1[base + j, sel] = s_sorted[np.where(sel, idxs, 0)][sel]
        datas.append(dict(node1=node1, perm=pc["perm"]))
    return datas, K, col0, ncols


def _pack_inputs(datas, gpos_of_node, parts, sent_pos):
    """Build per-core device input arrays from the slot plan."""
    per_core_inputs = []
    for c in range(NCORES):
        node1 = datas[c]["node1"]      # [ncols, 128], -1 invalid
        valid = node1 >= 0
        n1 = np.where(valid, node1, N)              # sentinel node 100000
        idx1_flat = (n1 // 4).astype(np.int16)
        oh1 = np.eye(4, dtype=np.int8)[n1 % 4]      # [ncols, 128, 4]

        g = np.where(valid, gpos_of_node[np.where(valid, node1, 0)],
                     sent_pos[c])
        idx2_flat = (g // 4).astype(np.int16)
        oh2 = np.eye(4, dtype=np.int8)[g % 4]

        w1l, w2l = [], []
        for (c0, nc_) in parts:
            w1l.append(_wrap_idx(idx1_flat[c0 : c0 + nc_].reshape(-1)))
            w2l.append(_wrap_idx(idx2_flat[c0 : c0 + nc_].reshape(-1)))
        per_core_inputs.append(dict(
            idx1=np.concatenate(w1l, axis=1),
            idx2=np.concatenate(w2l, axis=1),
            oh1=np.ascontiguousarray(oh1.transpose(1, 0, 2)),
            oh2=np.ascontiguousarray(oh2.transpose(1, 0, 2)),
        ))
    return per_core_inputs


_BUILD_CACHE = {}


def _build(K, col0, totc):
    import concourse.bass as bass
    import concourse.bacc as bacc
    import concourse.mybir as mybir
    import concourse.tile as tile
    from concourse.masks import make_identity

    f32 = mybir.dt.float32
    bf16 = mybir.dt.bfloat16
    i16 = mybir.dt.int16
    i32 = mybir.dt.int32
    i8 = mybir.dt.int8
    AX = mybir.AxisListType.X
    OP = mybir.AluOpType
    ACT = mybir.ActivationFunctionType

    ncols = int(K.sum())

    nc = bacc.Bacc("TRN2", target_bir_lowering=False, debug=False,
                   num_devices=NCORES, num_swdge_queues=4)

    x = nc.dram_tensor("x", [IN, NPAD], bf16, kind="ExternalInput")
    w1e = nc.dram_tensor("w1e", [IN, 80], bf16, kind="ExternalInput")
    w2e = nc.dram_tensor("w2e", [HID, 4], bf16, kind="ExternalInput")
    idx1 = nc.dram_tensor("idx1", [128, totc], i16, kind="ExternalInput")
    idx2 = nc.dram_tensor("idx2", [128, totc], i16, kind="ExternalInput")
    oh1 = nc.dram_tensor("oh1", [128, ncols, 4], i8, kind="ExternalInput")
    oh2 = nc.dram_tensor("oh2", [128, ncols, 4], i8, kind="ExternalInput")
    b1e = nc.dram_tensor("b1e", [128, HID], f32, kind="ExternalInput")
    b2e = nc.dram_tensor("b2e", [128, OUT], f32, kind="ExternalInput")

    table1 = nc.dram_tensor("table1", [T1ROWS, T1W], bf16, kind="Internal")
    t2shard = nc.dram_tensor("t2shard", [T2SROWS, T2W], bf16, kind="Internal")
    table2 = nc.dram_tensor("table2", [T2ROWS, T2W], bf16, kind="Internal",
                            addr_space="Shared")
    outp = nc.dram_tensor("outp", [PT, OUT], f32, kind="ExternalOutput")

    with tile.TileContext(nc) as tc:
        with (
            tc.tile_pool(name="const", bufs=1) as cpool,
            tc.tile_pool(name="node", bufs=3) as npool,
            tc.tile_pool(name="npsum", bufs=4, space="PSUM") as npsum,
            tc.tile_pool(name="gth", bufs=2) as gpool,
            tc.tile_pool(name="edge", bufs=3) as epool,
            tc.tile_pool(name="esmall", bufs=4) as espool,
            tc.tile_pool(name="slab", bufs=2) as slpool,
            tc.tile_pool(name="accs", bufs=2) as apool,
            tc.tile_pool(name="fin", bufs=2) as fpool,
            tc.tile_pool(name="fpsum", bufs=2, space="PSUM") as fpsum,
        ):
            ident = cpool.tile([128, 128], bf16)
            make_identity(nc, ident[:])
            w1es = cpool.tile([IN, 80], bf16)
            nc.sync.dma_start(out=w1es[:], in_=w1e[:])
            w2es = cpool.tile([HID, 4], bf16)
            nc.sync.dma_start(out=w2es[:], in_=w2e[:])
            b1es = cpool.tile([128, HID], f32)
            nc.sync.dma_start(out=b1es[:], in_=b1e[:])
            b2es = cpool.tile([128, OUT], f32)
            nc.sync.dma_start(out=b2es[:], in_=b2e[:])
            # persistent accumulators + staging
            accTn = cpool.tile([128, NT, HID], bf16)    # num (c-major)
            accTd = cpool.tile([128, NT, H1], f32)      # den
            accT2 = cpool.tile([128, NT, 3], f32)       # num(2)|den(1)
            zball = cpool.tile([128, NT, HID], bf16)    # elu output, c-major
            t2s = cpool.tile([128, NT, REC2], bf16)     # layer2 table staging
            t2raw = cpool.tile([128, NT, 4], f32)       # raw g2 results
            zrow = cpool.tile([128, T1W], bf16)         # sentinel zero row
            nc.vector.memset(zrow[:], 0.0)
            nc.vector.memset(t2s[:], 0.0)

            # ---- node phase: table1 records per node, 8 x-tiles per slab.
            # Host permutes x columns: slab col (i*128+p) holds node
            # sl*1024 + 8p + i, so the table write is a single strided DMA.
            for sl in range(NPAD // 1024):
                eng = nc.sync
                xs = npool.tile([IN, 1024], bf16, tag="xs")
                eng.dma_start(out=xs[:], in_=x[:, sl * 1024 : (sl + 1) * 1024])
                t1s = npool.tile([128, XS, REC1], bf16, tag="t1s")
                for hf in range(2):
                    g1 = npsum.tile([128, 4, 80], f32, tag="g1")
                    for i in range(4):
                        ti = hf * 4 + i
                        nc.tensor.matmul(
                            out=g1[:, i, :],
                            lhsT=xs[:, ti * 128 : (ti + 1) * 128],
                            rhs=w1es[:], start=True, stop=True)
                    sel = t1s[:, hf * 4 : hf * 4 + 4, :]
                    # h (already c-major from the W1 column permutation) + b1
                    nc.vector.tensor_tensor(
                        out=sel[:, :, 0:HID], in0=g1[:, :, 0:HID],
                        in1=b1es[:].unsqueeze(1).to_broadcast([128, 4, HID]),
                        op=OP.add)
                    nc.scalar.activation(out=sel[:, :, 64:72],
                                         in_=g1[:, :, 64:72], func=ACT.Exp)
                    nc.scalar.activation(out=sel[:, :, 72:80],
                                         in_=g1[:, :, 64:72], func=ACT.Exp,
                                         scale=NEG)
                    nc.vector.tensor_copy(out=sel[:, :, 80:88],
                                          in_=g1[:, :, 72:80])
                    nc.vector.memset(sel[:, :, 88:96], 0.0)
                # one strided DMA writes 1024 node records
                dst = table1[:].rearrange("r w -> (r w)").rearrange(
                    "(s p i1 i0 v) -> s p i1 i0 v",
                    p=128, i1=2, i0=4, v=REC1)[sl]
                eng.dma_start(out=dst, in_=t1s[:])
            # zero the sentinel row (node 100000)
            nc.sync.dma_start(out=table1[SENT_ROW : SENT_ROW + 1, :],
                              in_=zrow[0:1, :])

            # ---- edge phase runner
            def edge_phase(layer):
                if layer == 1:
                    idxT, ohT, tabT, EW, RC = idx1, oh1, table1, T1W, REC1
                    NH, NV = H1, HID
                    accn, accd = accTn, accTd
                else:
                    idxT, ohT, tabT, EW, RC = idx2, oh2, table2, T2W, REC2
                    NH, NV = 1, OUT
                    accn = accd = accT2
                RCI = RC // 2           # record in int32 elems
                SEL = (NV + 2 * NH) // 2  # int32 width of contiguous select
                # flat part list; idx/oh prefetched in 16-part slabs
                plist = []
                ioff = 0
                for t in range(NT):
                    kt_full = int(K[t])
                    for p0 in range(0, kt_full, PARTC):
                        kt = min(PARTC, kt_full - p0)
                        plist.append((t, p0, kt, ioff, int(col0[t]) + p0))
                        ioff += kt * 8

                SLAB = 16
                slab_tiles = {}

                def dispatch_slab(k):
                    lo = k * SLAB
                    hi = min(len(plist), lo + SLAB)
                    if lo >= len(plist):
                        return
                    i0 = plist[lo][3]
                    i1 = plist[hi - 1][3] + plist[hi - 1][2] * 8
                    c0 = plist[lo][4]
                    c1 = plist[hi - 1][4] + plist[hi - 1][2]
                    ixs = slpool.tile([128, SLAB * PARTC * 8], i16, tag="ixs")
                    nc.sync.dma_start(out=ixs[:, 0 : i1 - i0],
                                      in_=idxT[:, i0:i1])
                    ohs = slpool.tile([128, SLAB * PARTC, 4], i8, tag="ohs")
                    nc.sync.dma_start(out=ohs[:, 0 : c1 - c0, :],
                                      in_=ohT[:, c0:c1, :])
                    slab_tiles[k] = (ixs, ohs, i0, c0)

                dispatch_slab(0)
                gq = 0
                D1 = D2 = None
                for i, (t, p0, kt, ioff_, col) in enumerate(plist):
                    if True:
                        k = i // SLAB
                        if i % SLAB == 0:
                            dispatch_slab(k + 1)
                            slab_tiles.pop(k - 2, None)
                        first = p0 == 0
                        ixs, ohs, si0, sc0 = slab_tiles[k]
                        io8 = ioff_ - si0
                        oc = col - sc0
                        oh_t = ohs
                        gt = gpool.tile([128, PARTC, EW], bf16,
                                        tag=f"gt{layer}")
                        for cc in range(0, kt, CPC):
                            ncc = min(CPC, kt - cc)
                            nc.gpsimd.dma_gather(
                                gt[:, cc : cc + ncc, :], tabT[:],
                                ixs[:, io8 + cc * 8 : io8 + (cc + ncc) * 8],
                                ncc * 128, ncc * 128, EW, queue_num=gq % 4)
                            gq += 1
                        gti = gt[:, 0:kt, :].bitcast(i32)   # [128, kt, EW/2]
                        V = epool.tile([128, PARTC, NV + 2 * NH], bf16,
                                       tag=f"V{layer}")
                        Vi = V[:, 0:kt, :].bitcast(i32)
                        for s in range(4):
                            nc.vector.copy_predicated(
                                out=Vi,
                                mask=oh_t[:, oc : oc + kt, s : s + 1]
                                    .to_broadcast([128, kt, SEL]),
                                data=gti[:, :, s * RCI : s * RCI + SEL])
                        if first:
                            adt = espool.tile([128, 1, 2 * ((NH + 1) // 2)],
                                             bf16, tag=f"adt{layer}")
                            adti = adt[:].bitcast(i32)
                            na = adti.shape[-1]
                            for s in range(4):
                                nc.vector.copy_predicated(
                                    out=adti,
                                    mask=oh_t[:, oc : oc + 1, s : s + 1]
                                        .to_broadcast([128, 1, na]),
                                    data=gti[:, 0:1,
                                             s * RCI + SEL : s * RCI + SEL + na])
                            D1 = espool.tile([128, 1, NH], bf16,
                                            tag=f"D1{layer}")
                            D2 = espool.tile([128, 1, NH], bf16,
                                            tag=f"D2{layer}")
                            nc.scalar.activation(out=D1[:], in_=adt[:, :, 0:NH],
                                                 func=ACT.Exp)
                            nc.scalar.activation(out=D2[:], in_=adt[:, :, 0:NH],
                                                 func=ACT.Exp, scale=NEG)
                        m1 = espool.tile([128, PARTC, NH], bf16,
                                        tag=f"m1{layer}")
                        m2 = espool.tile([128, PARTC, NH], bf16,
                                        tag=f"m2{layer}")
                        w = espool.tile([128, PARTC, NH], bf16,
                                       tag=f"w{layer}")
                        nc.vector.tensor_tensor(
                            out=m1[:, 0:kt, :], in0=V[:, 0:kt, NV : NV + NH],
                            in1=D1[:].to_broadcast([128, kt, NH]), op=OP.mult)
                        nc.vector.tensor_tensor(
                            out=m2[:, 0:kt, :],
                            in0=V[:, 0:kt, NV + NH : NV + 2 * NH],
                            in1=D2[:].to_broadcast([128, kt, NH]), op=OP.mult)
                        nc.vector.tensor_tensor(
                            out=w[:, 0:kt, :], in0=m1[:, 0:kt, :],
                            in1=m2[:, 0:kt, :], op=OP.max)
                        Wj = epool.tile([128, PARTC, NV], bf16,
                                        tag=f"Wj{layer}")
                        if layer == 1:
                            # c-major V, weight broadcast on middle axis -> 2x
                            nc.vector.tensor_tensor(
                                out=Wj[:, 0:kt, :].rearrange(
                                    "p j (c h) -> p j c h", h=H1),
                                in0=V[:, 0:kt, 0:NV].rearrange(
                                    "p j (c h) -> p j c h", h=H1),
                                in1=w[:, 0:kt, :].unsqueeze(2).to_broadcast(
                                    [128, kt, C1, H1]),
                                op=OP.mult)
                        else:
                            nc.vector.tensor_tensor(
                                out=Wj[:, 0:kt, :], in0=V[:, 0:kt, 0:NV],
                                in1=w[:, 0:kt, :].to_broadcast([128, kt, NV]),
                                op=OP.mult)
                        nslice = (accn[:, t, 0:NV] if layer == 1
                                  else accn[:, t, 0:NV])
                        dslice = (accd[:, t, 0:NH] if layer == 1
                                  else accd[:, t, NV : NV + NH])
                        if first:
                            nc.vector.tensor_reduce(
                                out=nslice,
                                in_=Wj[:, 0:kt, :].rearrange("p j f -> p f j"),
                                axis=AX, op=OP.add)
                            nc.vector.tensor_reduce(
                                out=dslice,
                                in_=w[:, 0:kt, :].rearrange("p j h -> p h j"),
                                axis=AX, op=OP.add)
                        else:
                            rn = apool.tile([128, NV + NH], f32,
                                            tag=f"rn{layer}")
                            nc.vector.tensor_reduce(
                                out=rn[:, 0:NV],
                                in_=Wj[:, 0:kt, :].rearrange("p j f -> p f j"),
                                axis=AX, op=OP.add)
                            nc.vector.tensor_reduce(
                                out=rn[:, NV : NV + NH],
                                in_=w[:, 0:kt, :].rearrange("p j h -> p h j"),
                                axis=AX, op=OP.add)
                            nc.vector.tensor_tensor(
                                out=nslice, in0=nslice, in1=rn[:, 0:NV],
                                op=OP.add)
                            nc.vector.tensor_tensor(
                                out=dslice, in0=dslice,
                                in1=rn[:, NV : NV + NH], op=OP.add)

            with nc.allow_low_precision(reason="bf16 numerator accumulate"):
                edge_phase(1)

            # ---- batched layer-1 finalize: softmax divide + ELU, 14-tile chunks
            CH = 7
            for ch in range(NT // CH):
                den = fpool.tile([128, CH, H1], f32, tag="den")
                nc.vector.tensor_scalar(
                    out=den[:], in0=accTd[:, ch * CH : (ch + 1) * CH, :],
                    scalar1=1e-12, scalar2=None, op0=OP.max)
                rd = fpool.tile([128, CH, H1], f32, tag="rd")
                nc.vector.reciprocal(out=rd[:], in_=den[:])
                z = fpool.tile([128, CH, HID], f32, tag="z")
                nc.vector.tensor_tensor(
                    out=z[:].rearrange("p t (c h) -> p t c h", h=H1),
                    in0=accTn[:, ch * CH : (ch + 1) * CH, :].rearrange(
                        "p t (c h) -> p t c h", h=H1),
                    in1=rd[:].unsqueeze(2).to_broadcast([128, CH, C1, H1]),
                    op=OP.mult)
                # elu
                zp = fpool.tile([128, CH, HID], f32, tag="zp")
                nc.vector.tensor_scalar(out=zp[:], in0=z[:], scalar1=0.0,
                                        scalar2=None, op0=OP.max)
                nc.vector.tensor_scalar(out=z[:], in0=z[:], scalar1=0.0,
                                        scalar2=None, op0=OP.min)
                ez = fpool.tile([128, CH, HID], f32, tag="ez")
                nc.scalar.activation(out=ez[:], in_=z[:], func=ACT.Exp)
                nc.vector.tensor_scalar(out=ez[:], in0=ez[:], scalar1=-1.0,
                                        scalar2=None, op0=OP.add)
                nc.vector.tensor_tensor(
                    out=zball[:, ch * CH : (ch + 1) * CH, :], in0=zp[:],
                    in1=ez[:], op=OP.add)

            # ---- layer-2 projection per tile + staging
            for t in range(NT):
                zT_ps = fpsum.tile([HID, 128], bf16, tag="zTp")
                nc.tensor.transpose(out=zT_ps[:], in_=zball[:, t, :],
                                    identity=ident[:])
                zTs = fpool.tile([HID, 128], bf16, tag="zTs")
                nc.vector.tensor_copy(out=zTs[:], in_=zT_ps[:])
                g2 = fpsum.tile([128, 4], f32, tag="g2p")
                nc.tensor.matmul(out=g2[:], lhsT=zTs[:], rhs=w2es[:],
                                 start=True, stop=True)
                nc.vector.tensor_copy(out=t2raw[:, t, :], in_=g2[:])
            # batched staging ops
            nc.vector.tensor_tensor(
                out=t2s[:, :, 0:OUT], in0=t2raw[:, :, 0:OUT],
                in1=b2es[:].unsqueeze(1).to_broadcast([128, NT, OUT]),
                op=OP.add)
            nc.scalar.activation(out=t2s[:, :, 2:3], in_=t2raw[:, :, 2:3],
                                 func=ACT.Exp)
            nc.scalar.activation(out=t2s[:, :, 3:4], in_=t2raw[:, :, 2:3],
                                 func=ACT.Exp, scale=NEG)
            nc.vector.tensor_copy(out=t2s[:, :, 4:5], in_=t2raw[:, :, 3:4])
            # single strided DMA: slot (p, t) -> position p*98+t
            t2dst = t2shard[0 : PT // 4, :].rearrange("r w -> (r w)").rearrange(
                "(p t v) -> p t v", p=128, t=NT)
            nc.sync.dma_start(out=t2dst, in_=t2s[:])
            # zero sentinel row (gathered by invalid layer-2 slots)
            nc.sync.dma_start(out=t2shard[PT // 4 : T2SROWS, :],
                              in_=zrow[0:1, 0:T2W])

            # ---- exchange layer-2 node features
            nc.gpsimd.collective_compute(
                "AllGather",
                OP.bypass,
                replica_groups=[list(range(NCORES))],
                ins=[t2shard[:]],
                outs=[table2[:]],
            )

            with nc.allow_low_precision(reason="bf16 numerator accumulate"):
                edge_phase(2)

            # ---- batched layer-2 finalize
            den2 = fpool.tile([128, NT, 1], f32, tag="den2")
            nc.vector.tensor_scalar(out=den2[:], in0=accT2[:, :, 2:3],
                                    scalar1=1e-12, scalar2=None, op0=OP.max)
            rd2 = fpool.tile([128, NT, 1], f32, tag="rd2")
            nc.vector.reciprocal(out=rd2[:], in_=den2[:])
            o2 = fpool.tile([128, NT, OUT], f32, tag="o2")
            nc.vector.tensor_tensor(
                out=o2[:], in0=accT2[:, :, 0:OUT],
                in1=rd2[:].to_broadcast([128, NT, OUT]), op=OP.mult)
            odst = outp[:].rearrange("r w -> (r w)").rearrange(
                "(p t v) -> p t v", p=128, t=NT)
            nc.sync.dma_start(out=odst, in_=o2[:])

    nc.compile()
    return nc


def kernel(**inputs):
    from concourse.bass_utils import run_bass_kernel_spmd

    x = np.asarray(inputs["x"], dtype=np.float32)
    ei = np.asarray(inputs["edge_index"]).astype(np.int64)
    w1 = np.asarray(inputs["W1"], dtype=np.float32)
    a1s = np.asarray(inputs["a1_src"], dtype=np.float32)
    a1d = np.asarray(inputs["a1_dst"], dtype=np.float32)
    b1 = np.asarray(inputs["b1"], dtype=np.float32)
    w2 = np.asarray(inputs["W2"], dtype=np.float32)
    a2s = np.asarray(inputs["a2_src"], dtype=np.float32)
    a2d = np.asarray(inputs["a2_dst"], dtype=np.float32)
    b2 = np.asarray(inputs["b2"], dtype=np.float32)

    src = ei[0]
    dst = ei[1]

    datas, K, col0, ncols = _plan(src, dst)
    parts = []
    for t in range(NT):
        for c in range(0, int(K[t]), PARTC):
            parts.append((int(col0[t]) + c, min(PARTC, int(K[t]) - c)))
    totc = sum(nc_ * 8 for _, nc_ in parts)

    # global position of each node for the L2 table: pos = c*4*T2SROWS +
    # p*98 + t where the node is dst slot (t, p) on core c.
    gpos_of_node = np.zeros(N, dtype=np.int64)
    s_old = np.arange(PT)
    tt = s_old // 128
    pp = s_old % 128
    pos_of_slot = pp * NT + tt
    for c in range(NCORES):
        perm = datas[c]["perm"]  # [PT] local dst ids (or -1)
        real = perm >= 0
        gpos_of_node[c * ND + perm[real]] = c * 4 * T2SROWS + pos_of_slot[real]
    sent_pos = np.array([c * 4 * T2SROWS + PT for c in range(NCORES)],
                        dtype=np.int64)

    per_core = _pack_inputs(datas, gpos_of_node, parts, sent_pos)

    # weights: w1e = [W1 | W1@A1s | W1@A1d], h block c-major downstream
    A1s = np.zeros((HID, H1), dtype=np.float32)
    A1d = np.zeros((HID, H1), dtype=np.float32)
    for h in range(H1):
        A1s[h * C1 : (h + 1) * C1, h] = a1s[h]
        A1d[h * C1 : (h + 1) * C1, h] = a1d[h]
    # h block emitted c-major directly: permute W1 columns (and b1)
    cm = (np.arange(HID) % H1) * C1 + (np.arange(HID) // H1)
    w1e = np.concatenate([w1[:, cm], w1 @ A1s, w1 @ A1d], axis=1)   # [128, 80]
    w2cm = w2[cm]
    w2e = np.concatenate([w2cm, w2cm @ a2s.T, w2cm @ a2d.T], axis=1)  # [64, 4]
    b1e = np.tile(b1[cm][None, :], (128, 1)).astype(np.float32)
    b2e = np.tile(b2[None, :], (128, 1)).astype(np.float32)

    # x: pad and permute columns (slab col i*128+p holds node sl*1024+8p+i)
    xp = np.zeros((NPAD, IN), dtype=np.float32)
    xp[:N] = x
    j = np.arange(1024)
    perm1024 = 8 * (j % 128) + j // 128
    permall = (np.arange(NPAD).reshape(-1, 1024) // 1024) * 1024
    permall = (permall + perm1024[None, :]).reshape(-1)
    xpT = np.ascontiguousarray(xp[permall].T.astype(BF16))

    key = (totc, tuple(K.tolist()))
    if key not in _BUILD_CACHE:
        _BUILD_CACHE[key] = _build(K, col0, totc)
    nc = _BUILD_CACHE[key]

    common = dict(x=xpT, w1e=w1e.astype(BF16), w2e=w2e.astype(BF16),
                  b1e=b1e, b2e=b2e)
    in_maps = []
    for c in range(NCORES):
        m = dict(common)
        m.update(per_core[c])
        in_maps.append(m)

    global _LAST_IN_MAPS
    _LAST_IN_MAPS = in_maps
    res = run_bass_kernel_spmd(nc, in_maps, list(range(NCORES)))

    out = np.zeros((N, OUT), dtype=np.float32)
    for c in range(NCORES):
        op = res.results[c]["outp"]       # [PT, 2], row = p*98+t
        perm = datas[c]["perm"]
        real = perm >= 0
        out[c * ND + perm[real]] = op[pos_of_slot[real]]
    return out


# revision 15
# speedup vs baseline: 1.2991x; 1.1630x over previous
"""GAT (2-layer, PyG-style) on 8 Trainium2 NeuronCores via Bass/Tile.

Strategy (dst-sharded, degree-sorted tiles), v2:
- Nodes sharded by dst across 8 cores (12500 each). Per core, dsts are
  degree-sorted and grouped into 98 tiles of 128 (partition = dst).
- Per tile, column 0 gathers the dst's own table row (serving both the
  self-loop edge and al_dst); remaining columns hold in-edges; invalid
  slots gather a zeroed sentinel row so no masking stream is needed.
- Layer tables are 4-node-packed rows (<=32767 rows, int16 dma_gather
  indices); the 4-way select runs as int32-pair predicated copies (half
  the DVE elements of a bf16 select).
- exp(LeakyReLU(a+b)) == max(exp(a)exp(b), exp(a/5)exp(b/5)^...) trick:
  tables store E1=exp(al_src), E2=exp(0.2*al_src); per-tile D1=exp(al_dst),
  D2=exp(0.2*al_dst); per-edge weight w = max(E1*D1, E2*D2). No per-edge
  exp and no LeakyReLU on the hot path.
- h stored c-major so the alpha*h multiply runs in the DVE 2x packed mode
  with the weight broadcast on a middle axis.
- Node phase processes 8 x-tiles per DMA; host pre-permutes x columns so
  each slab writes table1 with a single strided DMA.
- Per-tile accumulators land in a persistent SBUF buffer; softmax
  normalization, ELU and the layer-2 projection run as batched ops after
  the edge phase (no per-tile serial finalize chain).
- Layer-2 per-node features are exchanged via an on-chip AllGather of
  4-packed shards in core-local permuted order.
"""

import numpy as np
import ml_dtypes

BF16 = ml_dtypes.bfloat16

N = 100_000
E = 3_200_000
IN = 128
H1, C1 = 8, 8
HID = H1 * C1          # 64
OUT = 2
NEG = 0.2
NCORES = 8
ND = N // NCORES       # dsts per core: 12500
NT = 98                # tiles per core (98*128 = 12544)
PT = NT * 128          # padded dst slots per core
NPAD = 100_352         # x padded to 98*1024
T1ROWS = NPAD // 4     # 25088 4-pack rows
REC1 = 96              # elems per node record: h(64)|E1(8)|E2(8)|aldst(8)|pad
T1W = 4 * REC1         # 384 bf16 per table1 row (768B)
SENT_ROW = 25_000      # row holding node 100000 (zeroed sentinel)
T2SROWS = PT // 4 + 1  # per-core layer2 shard rows (+1 zero sentinel row)
T2ROWS = T2SROWS * NCORES     # 25096
REC2 = 32              # z0 z1 E1 E2 aldst 0...
T2W = 4 * REC2         # 128 bf16 per table2 row (256B)
PARTC = 24             # columns per tile-part
CPC = 8                # columns per dma_gather call (65 descs; ring holds one
                       # in-flight call per queue -- larger calls wedge)
XS = 8                 # x-tiles per node-phase slab


def _wrap_idx(flat):
    """int16 index array -> [128, n/16] wrapped-in-16-partitions, replicated x8."""
    n = flat.shape[0]
    assert n % 16 == 0
    w = flat.reshape(n // 16, 16).T            # [16, n/16]
    return np.tile(w, (8, 1)).astype(np.int16)  # [128, n/16]


def _plan(src, dst):
    """Host-side index planning. Returns per-core data arrays + common schedule."""
    core = dst // ND
    dloc = dst % ND

    per_core = []
    for c in range(NCORES):
        m = core == c
        s_c = src[m]
        d_c = dloc[m]
        deg = np.bincount(d_c, minlength=ND)  # in-edges, no self loop yet
        order = np.argsort(-deg, kind="stable")  # degree-desc permutation
        perm = np.full(PT, -1, dtype=np.int64)
        perm[:ND] = order
        degp = np.zeros(PT, dtype=np.int64)
        degp[:ND] = deg[order]
        # group in-edges by dst for slot filling
        sort_by_d = np.argsort(d_c, kind="stable")
        s_sorted = s_c[sort_by_d]
        starts = np.zeros(ND + 1, dtype=np.int64)
        np.cumsum(deg, out=starts[1:])
        per_core.append(dict(perm=perm, degp=degp, s_sorted=s_sorted, starts=starts))

    # common K_t schedule: columns per tile = 1 (self/dst col) + max in-degree
    K = np.zeros(NT, dtype=np.int64)
    for t in range(NT):
        mx = 0
        for c in range(NCORES):
            d = per_core[c]["degp"][t * 128 : (t + 1) * 128]
            mx = max(mx, int(d.max()) if d.size else 0)
        K[t] = mx + 1
    col0 = np.zeros(NT, dtype=np.int64)
    pos = 0
    for t in range(NT):
        col0[t] = pos
        pos += K[t]
    ncols = pos

    # per-core slot arrays: node id per (col, partition); -1 = invalid
    datas = []
    for c in range(NCORES):
        pc = per_core[c]
        perm, degp, s_sorted, starts = (
            pc["perm"], pc["degp"], pc["s_sorted"], pc["starts"],
        )
        node1 = np.full((ncols, 128), -1, dtype=np.int64)
        for t in range(NT):
            base = col0[t]
            d_orig = perm[t * 128 : (t + 1) * 128]           # local dst ids, -1 pad
            real = d_orig >= 0
            dg = np.where(real, d_orig, 0)
            node1[base, real] = (c * ND + dg)[real]
            kt = int(K[t])
            if kt > 1:
                st = starts[dg]
                cnt = degp[t * 128 : (t + 1) * 128]
                for j in range(1, kt):
                    sel = (j - 1 < cnt) & real
                    idxs = st + (j - 1)
                    node1[base + j, sel] = s_sorted[np.where(sel, idxs, 0)][sel]
        datas.append(dict(node1=node1, perm=pc["perm"]))
    return datas, K, col0, ncols


def _pack_inputs(datas, gpos_of_node, parts, sent_pos):
    """Build per-core device input arrays from the slot plan."""
    per_core_inputs = []
    for c in range(NCORES):
        node1 = datas[c]["node1"]      # [ncols, 128], -1 invalid
        valid = node1 >= 0
        n1 = np.where(valid, node1, N)              # sentinel node 100000
        idx1_flat = (n1 // 4).astype(np.int16)
        oh1 = np.eye(4, dtype=np.int8)[n1 % 4]      # [ncols, 128, 4]

        g = np.where(valid, gpos_of_node[np.where(valid, node1, 0)],
                     sent_pos[c])
        idx2_flat = (g // 4).astype(np.int16)
        oh2 = np.eye(4, dtype=np.int8)[g % 4]

        w1l, w2l = [], []
        for (c0, nc_) in parts:
            w1l.append(_wrap_idx(idx1_flat[c0 : c0 + nc_].reshape(-1)))
            w2l.append(_wrap_idx(idx2_flat[c0 : c0 + nc_].reshape(-1)))
        per_core_inputs.append(dict(
            idx1=np.concatenate(w1l, axis=1),
            idx2=np.concatenate(w2l, axis=1),
            oh1=np.ascontiguousarray(oh1.transpose(1, 0, 2)),
            oh2=np.ascontiguousarray(oh2.transpose(1, 0, 2)),
        ))
    return per_core_inputs


_BUILD_CACHE = {}


def _build(K, col0, totc):
    import concourse.bass as bass
    import concourse.bacc as bacc
    import concourse.mybir as mybir
    import concourse.tile as tile
    from concourse.masks import make_identity

    f32 = mybir.dt.float32
    bf16 = mybir.dt.bfloat16
    i16 = mybir.dt.int16
    i32 = mybir.dt.int32
    i8 = mybir.dt.int8
    AX = mybir.AxisListType.X
    OP = mybir.AluOpType
    ACT = mybir.ActivationFunctionType

    ncols = int(K.sum())

    nc = bacc.Bacc("TRN2", target_bir_lowering=False, debug=False,
                   num_devices=NCORES, num_swdge_queues=4)

    x = nc.dram_tensor("x", [IN, NPAD], bf16, kind="ExternalInput")
    w1e = nc.dram_tensor("w1e", [IN, 80], bf16, kind="ExternalInput")
    w2e = nc.dram_tensor("w2e", [HID, 4], bf16, kind="ExternalInput")
    idx1 = nc.dram_tensor("idx1", [128, totc], i16, kind="ExternalInput")
    idx2 = nc.dram_tensor("idx2", [128, totc], i16, kind="ExternalInput")
    oh1 = nc.dram_tensor("oh1", [128, ncols, 4], i8, kind="ExternalInput")
    oh2 = nc.dram_tensor("oh2", [128, ncols, 4], i8, kind="ExternalInput")
    b1e = nc.dram_tensor("b1e", [128, HID], f32, kind="ExternalInput")
    b2e = nc.dram_tensor("b2e", [128, OUT], f32, kind="ExternalInput")

    table1 = nc.dram_tensor("table1", [T1ROWS, T1W], bf16, kind="Internal")
    t2shard = nc.dram_tensor("t2shard", [T2SROWS, T2W], bf16, kind="Internal")
    table2 = nc.dram_tensor("table2", [T2ROWS, T2W], bf16, kind="Internal",
                            addr_space="Shared")
    outp = nc.dram_tensor("outp", [PT, OUT], f32, kind="ExternalOutput")

    with tile.TileContext(nc) as tc:
        with (
            tc.tile_pool(name="const", bufs=1) as cpool,
            tc.tile_pool(name="node", bufs=3) as npool,
            tc.tile_pool(name="npsum", bufs=4, space="PSUM") as npsum,
            tc.tile_pool(name="gth", bufs=3) as gpool,
            tc.tile_pool(name="edge", bufs=3) as epool,
            tc.tile_pool(name="esmall", bufs=4) as espool,
            tc.tile_pool(name="slab", bufs=2) as slpool,
            tc.tile_pool(name="accs", bufs=2) as apool,
            tc.tile_pool(name="fin", bufs=2) as fpool,
            tc.tile_pool(name="fpsum", bufs=2, space="PSUM") as fpsum,
        ):
            ident = cpool.tile([128, 128], bf16)
            make_identity(nc, ident[:])
            w1es = cpool.tile([IN, 80], bf16)
            nc.sync.dma_start(out=w1es[:], in_=w1e[:])
            w2es = cpool.tile([HID, 4], bf16)
            nc.sync.dma_start(out=w2es[:], in_=w2e[:])
            b1es = cpool.tile([128, HID], f32)
            nc.sync.dma_start(out=b1es[:], in_=b1e[:])
            b2es = cpool.tile([128, OUT], f32)
            nc.sync.dma_start(out=b2es[:], in_=b2e[:])
            # persistent accumulators + staging
            accTn = cpool.tile([128, NT, HID], bf16)    # num (c-major)
            accTd = cpool.tile([128, NT, H1], f32)      # den
            accT2 = cpool.tile([128, NT, 3], f32)       # num(2)|den(1)
            zball = cpool.tile([128, NT, HID], bf16)    # elu output, c-major
            t2s = cpool.tile([128, NT, REC2], bf16)     # layer2 table staging
            t2raw = cpool.tile([128, NT, 4], f32)       # raw g2 results
            zrow = cpool.tile([128, T1W], bf16)         # sentinel zero row
            nc.vector.memset(zrow[:], 0.0)
            nc.vector.memset(t2s[:], 0.0)

            # ---- node phase: table1 records per node, 8 x-tiles per slab.
            # Host permutes x columns: slab col (i*128+p) holds node
            # sl*1024 + 8p + i, so the table write is a single strided DMA.
            for sl in range(NPAD // 1024):
                eng = nc.sync
                xs = npool.tile([IN, 1024], bf16, tag="xs")
                eng.dma_start(out=xs[:], in_=x[:, sl * 1024 : (sl + 1) * 1024])
                t1s = npool.tile([128, XS, REC1], bf16, tag="t1s")
                for hf in range(2):
                    g1 = npsum.tile([128, 4, 80], f32, tag="g1")
                    for i in range(4):
                        ti = hf * 4 + i
                        nc.tensor.matmul(
                            out=g1[:, i, :],
                            lhsT=xs[:, ti * 128 : (ti + 1) * 128],
                            rhs=w1es[:], start=True, stop=True)
                    sel = t1s[:, hf * 4 : hf * 4 + 4, :]
                    # h (already c-major from the W1 column permutation) + b1
                    nc.vector.tensor_tensor(
                        out=sel[:, :, 0:HID], in0=g1[:, :, 0:HID],
                        in1=b1es[:].unsqueeze(1).to_broadcast([128, 4, HID]),
                        op=OP.add)
                    nc.scalar.activation(out=sel[:, :, 64:72],
                                         in_=g1[:, :, 64:72], func=ACT.Exp)
                    nc.scalar.activation(out=sel[:, :, 72:80],
                                         in_=g1[:, :, 64:72], func=ACT.Exp,
                                         scale=NEG)
                    nc.vector.tensor_copy(out=sel[:, :, 80:88],
                                          in_=g1[:, :, 72:80])
                    nc.vector.memset(sel[:, :, 88:96], 0.0)
                # one strided DMA writes 1024 node records
                dst = table1[:].rearrange("r w -> (r w)").rearrange(
                    "(s p i1 i0 v) -> s p i1 i0 v",
                    p=128, i1=2, i0=4, v=REC1)[sl]
                eng.dma_start(out=dst, in_=t1s[:])
            # zero the sentinel row (node 100000)
            nc.sync.dma_start(out=table1[SENT_ROW : SENT_ROW + 1, :],
                              in_=zrow[0:1, :])

            # ---- edge phase runner
            def edge_phase(layer):
                if layer == 1:
                    idxT, ohT, tabT, EW, RC = idx1, oh1, table1, T1W, REC1
                    NH, NV = H1, HID
                    accn, accd = accTn, accTd
                else:
                    idxT, ohT, tabT, EW, RC = idx2, oh2, table2, T2W, REC2
                    NH, NV = 1, OUT
                    accn = accd = accT2
                RCI = RC // 2           # record in int32 elems
                SEL = (NV + 2 * NH) // 2  # int32 width of contiguous select
                # flat part list; idx/oh prefetched in 16-part slabs
                plist = []
                ioff = 0
                for t in range(NT):
                    kt_full = int(K[t])
                    for p0 in range(0, kt_full, PARTC):
                        kt = min(PARTC, kt_full - p0)
                        plist.append((t, p0, kt, ioff, int(col0[t]) + p0))
                        ioff += kt * 8

                SLAB = 16
                slab_tiles = {}

                def dispatch_slab(k):
                    lo = k * SLAB
                    hi = min(len(plist), lo + SLAB)
                    if lo >= len(plist):
                        return
                    i0 = plist[lo][3]
                    i1 = plist[hi - 1][3] + plist[hi - 1][2] * 8
                    c0 = plist[lo][4]
                    c1 = plist[hi - 1][4] + plist[hi - 1][2]
                    ixs = slpool.tile([128, SLAB * PARTC * 8], i16, tag="ixs")
                    nc.sync.dma_start(out=ixs[:, 0 : i1 - i0],
                                      in_=idxT[:, i0:i1])
                    ohs = slpool.tile([128, SLAB * PARTC, 4], i8, tag="ohs")
                    nc.sync.dma_start(out=ohs[:, 0 : c1 - c0, :],
                                      in_=ohT[:, c0:c1, :])
                    slab_tiles[k] = (ixs, ohs, i0, c0)

                dispatch_slab(0)
                gq = 0
                D1 = D2 = None
                for i, (t, p0, kt, ioff_, col) in enumerate(plist):
                    if True:
                        k = i // SLAB
                        if i % SLAB == 0:
                            dispatch_slab(k + 1)
                            slab_tiles.pop(k - 2, None)
                        first = p0 == 0
                        ixs, ohs, si0, sc0 = slab_tiles[k]
                        io8 = ioff_ - si0
                        oc = col - sc0
                        oh_t = ohs
                        gt = gpool.tile([128, PARTC, EW], bf16,
                                        tag=f"gt{layer}")
                        for cc in range(0, kt, CPC):
                            ncc = min(CPC, kt - cc)
                            nc.gpsimd.dma_gather(
                                gt[:, cc : cc + ncc, :], tabT[:],
                                ixs[:, io8 + cc * 8 : io8 + (cc + ncc) * 8],
                                ncc * 128, ncc * 128, EW, queue_num=gq % 4)
                            gq += 1
                        gti = gt[:, 0:kt, :].bitcast(i32)   # [128, kt, EW/2]
                        V = epool.tile([128, PARTC, NV + 2 * NH], bf16,
                                       tag=f"V{layer}")
                        Vi = V[:, 0:kt, :].bitcast(i32)
                        for s in range(4):
                            nc.vector.copy_predicated(
                                out=Vi,
                                mask=oh_t[:, oc : oc + kt, s : s + 1]
                                    .to_broadcast([128, kt, SEL]),
                                data=gti[:, :, s * RCI : s * RCI + SEL])
                        if first:
                            adt = espool.tile([128, 1, 2 * ((NH + 1) // 2)],
                                             bf16, tag=f"adt{layer}")
                            adti = adt[:].bitcast(i32)
                            na = adti.shape[-1]
                            for s in range(4):
                                nc.vector.copy_predicated(
                                    out=adti,
                                    mask=oh_t[:, oc : oc + 1, s : s + 1]
                                        .to_broadcast([128, 1, na]),
                                    data=gti[:, 0:1,
                                             s * RCI + SEL : s * RCI + SEL + na])
                            D1 = espool.tile([128, 1, NH], bf16,
                                            tag=f"D1{layer}")
                            D2 = espool.tile([128, 1, NH], bf16,
                                            tag=f"D2{layer}")
                            nc.scalar.activation(out=D1[:], in_=adt[:, :, 0:NH],
                                                 func=ACT.Exp)
                            nc.scalar.activation(out=D2[:], in_=adt[:, :, 0:NH],
                                                 func=ACT.Exp, scale=NEG)
                        m1 = espool.tile([128, PARTC, NH], bf16,
                                        tag=f"m1{layer}")
                        m2 = espool.tile([128, PARTC, NH], bf16,
                                        tag=f"m2{layer}")
                        w = espool.tile([128, PARTC, NH], bf16,
                                       tag=f"w{layer}")
                        nc.vector.tensor_tensor(
                            out=m1[:, 0:kt, :], in0=V[:, 0:kt, NV : NV + NH],
                            in1=D1[:].to_broadcast([128, kt, NH]), op=OP.mult)
                        nc.vector.tensor_tensor(
                            out=m2[:, 0:kt, :],
                            in0=V[:, 0:kt, NV + NH : NV + 2 * NH],
                            in1=D2[:].to_broadcast([128, kt, NH]), op=OP.mult)
                        nc.vector.tensor_tensor(
                            out=w[:, 0:kt, :], in0=m1[:, 0:kt, :],
                            in1=m2[:, 0:kt, :], op=OP.max)
                        Wj = epool.tile([128, PARTC, NV], bf16,
                                        tag=f"Wj{layer}")
                        if layer == 1:
                            # c-major V, weight broadcast on middle axis -> 2x
                            nc.vector.tensor_tensor(
                                out=Wj[:, 0:kt, :].rearrange(
                                    "p j (c h) -> p j c h", h=H1),
                                in0=V[:, 0:kt, 0:NV].rearrange(
                                    "p j (c h) -> p j c h", h=H1),
                                in1=w[:, 0:kt, :].unsqueeze(2).to_broadcast(
                                    [128, kt, C1, H1]),
                                op=OP.mult)
                        else:
                            nc.vector.tensor_tensor(
                                out=Wj[:, 0:kt, :], in0=V[:, 0:kt, 0:NV],
                                in1=w[:, 0:kt, :].to_broadcast([128, kt, NV]),
                                op=OP.mult)
                        nslice = (accn[:, t, 0:NV] if layer == 1
                                  else accn[:, t, 0:NV])
                        dslice = (accd[:, t, 0:NH] if layer == 1
                                  else accd[:, t, NV : NV + NH])
                        if first:
                            nc.vector.tensor_reduce(
                                out=nslice,
                                in_=Wj[:, 0:kt, :].rearrange("p j f -> p f j"),
                                axis=AX, op=OP.add)
                            nc.vector.tensor_reduce(
                                out=dslice,
                                in_=w[:, 0:kt, :].rearrange("p j h -> p h j"),
                                axis=AX, op=OP.add)
                        else:
                            rn = apool.tile([128, NV + NH], f32,
                                            tag=f"rn{layer}")
                            nc.vector.tensor_reduce(
                                out=rn[:, 0:NV],
                                in_=Wj[:, 0:kt, :].rearrange("p j f -> p f j"),
                                axis=AX, op=OP.add)
                            nc.vector.tensor_reduce(
                                out=rn[:, NV : NV + NH],
                                in_=w[:, 0:kt, :].rearrange("p j h -> p h j"),
                                axis=AX, op=OP.add)
                            nc.vector.tensor_tensor(
                                out=nslice, in0=nslice, in1=rn[:, 0:NV],
                                op=OP.add)
                            nc.vector.tensor_tensor(
                                out=dslice, in0=dslice,
                                in1=rn[:, NV : NV + NH], op=OP.add)

            with nc.allow_low_precision(reason="bf16 numerator accumulate"):
                edge_phase(1)

            # ---- batched layer-1 finalize: softmax divide + ELU, 14-tile chunks
            CH = 7
            for ch in range(NT // CH):
                den = fpool.tile([128, CH, H1], f32, tag="den")
                nc.vector.tensor_scalar(
                    out=den[:], in0=accTd[:, ch * CH : (ch + 1) * CH, :],
                    scalar1=1e-12, scalar2=None, op0=OP.max)
                rd = fpool.tile([128, CH, H1], f32, tag="rd")
                nc.vector.reciprocal(out=rd[:], in_=den[:])
                z = fpool.tile([128, CH, HID], f32, tag="z")
                nc.vector.tensor_tensor(
                    out=z[:].rearrange("p t (c h) -> p t c h", h=H1),
                    in0=accTn[:, ch * CH : (ch + 1) * CH, :].rearrange(
                        "p t (c h) -> p t c h", h=H1),
                    in1=rd[:].unsqueeze(2).to_broadcast([128, CH, C1, H1]),
                    op=OP.mult)
                # elu
                zp = fpool.tile([128, CH, HID], f32, tag="zp")
                nc.vector.tensor_scalar(out=zp[:], in0=z[:], scalar1=0.0,
                                        scalar2=None, op0=OP.max)
                nc.vector.tensor_scalar(out=z[:], in0=z[:], scalar1=0.0,
                                        scalar2=None, op0=OP.min)
                ez = fpool.tile([128, CH, HID], f32, tag="ez")
                nc.scalar.activation(out=ez[:], in_=z[:], func=ACT.Exp)
                nc.vector.tensor_scalar(out=ez[:], in0=ez[:], scalar1=-1.0,
                                        scalar2=None, op0=OP.add)
                nc.vector.tensor_tensor(
                    out=zball[:, ch * CH : (ch + 1) * CH, :], in0=zp[:],
                    in1=ez[:], op=OP.add)

            # ---- layer-2 projection per tile + staging
            for t in range(NT):
                zT_ps = fpsum.tile([HID, 128], bf16, tag="zTp")
                nc.tensor.transpose(out=zT_ps[:], in_=zball[:, t, :],
                                    identity=ident[:])
                zTs = fpool.tile([HID, 128], bf16, tag="zTs")
                nc.vector.tensor_copy(out=zTs[:], in_=zT_ps[:])
                g2 = fpsum.tile([128, 4], f32, tag="g2p")
                nc.tensor.matmul(out=g2[:], lhsT=zTs[:], rhs=w2es[:],
                                 start=True, stop=True)
                nc.vector.tensor_copy(out=t2raw[:, t, :], in_=g2[:])
            # batched staging ops
            nc.vector.tensor_tensor(
                out=t2s[:, :, 0:OUT], in0=t2raw[:, :, 0:OUT],
                in1=b2es[:].unsqueeze(1).to_broadcast([128, NT, OUT]),
                op=OP.add)
            nc.scalar.activation(out=t2s[:, :, 2:3], in_=t2raw[:, :, 2:3],
                                 func=ACT.Exp)
            nc.scalar.activation(out=t2s[:, :, 3:4], in_=t2raw[:, :, 2:3],
                                 func=ACT.Exp, scale=NEG)
            nc.vector.tensor_copy(out=t2s[:, :, 4:5], in_=t2raw[:, :, 3:4])
            # single strided DMA: slot (p, t) -> position p*98+t
            t2dst = t2shard[0 : PT // 4, :].rearrange("r w -> (r w)").rearrange(
                "(p t v) -> p t v", p=128, t=NT)
            nc.sync.dma_start(out=t2dst, in_=t2s[:])
            # zero sentinel row (gathered by invalid layer-2 slots)
            nc.sync.dma_start(out=t2shard[PT // 4 : T2SROWS, :],
                              in_=zrow[0:1, 0:T2W])

            # ---- exchange layer-2 node features
            nc.gpsimd.collective_compute(
                "AllGather",
                OP.bypass,
                replica_groups=[list(range(NCORES))],
                ins=[t2shard[:]],
                outs=[table2[:]],
            )

            with nc.allow_low_precision(reason="bf16 numerator accumulate"):
                edge_phase(2)

            # ---- batched layer-2 finalize
            den2 = fpool.tile([128, NT, 1], f32, tag="den2")
            nc.vector.tensor_scalar(out=den2[:], in0=accT2[:, :, 2:3],
                                    scalar1=1e-12, scalar2=None, op0=OP.max)
            rd2 = fpool.tile([128, NT, 1], f32, tag="rd2")
            nc.vector.reciprocal(out=rd2[:], in_=den2[:])
            o2 = fpool.tile([128, NT, OUT], f32, tag="o2")
            nc.vector.tensor_tensor(
                out=o2[:], in0=accT2[:, :, 0:OUT],
                in1=rd2[:].to_broadcast([128, NT, OUT]), op=OP.mult)
            odst = outp[:].rearrange("r w -> (r w)").rearrange(
                "(p t v) -> p t v", p=128, t=NT)
            nc.sync.dma_start(out=odst, in_=o2[:])

    nc.compile()
    return nc


def kernel(**inputs):
    from concourse.bass_utils import run_bass_kernel_spmd

    x = np.asarray(inputs["x"], dtype=np.float32)
    ei = np.asarray(inputs["edge_index"]).astype(np.int64)
    w1 = np.asarray(inputs["W1"], dtype=np.float32)
    a1s = np.asarray(inputs["a1_src"], dtype=np.float32)
    a1d = np.asarray(inputs["a1_dst"], dtype=np.float32)
    b1 = np.asarray(inputs["b1"], dtype=np.float32)
    w2 = np.asarray(inputs["W2"], dtype=np.float32)
    a2s = np.asarray(inputs["a2_src"], dtype=np.float32)
    a2d = np.asarray(inputs["a2_dst"], dtype=np.float32)
    b2 = np.asarray(inputs["b2"], dtype=np.float32)

    src = ei[0]
    dst = ei[1]

    datas, K, col0, ncols = _plan(src, dst)
    parts = []
    for t in range(NT):
        for c in range(0, int(K[t]), PARTC):
            parts.append((int(col0[t]) + c, min(PARTC, int(K[t]) - c)))
    totc = sum(nc_ * 8 for _, nc_ in parts)

    # global position of each node for the L2 table: pos = c*4*T2SROWS +
    # p*98 + t where the node is dst slot (t, p) on core c.
    gpos_of_node = np.zeros(N, dtype=np.int64)
    s_old = np.arange(PT)
    tt = s_old // 128
    pp = s_old % 128
    pos_of_slot = pp * NT + tt
    for c in range(NCORES):
        perm = datas[c]["perm"]  # [PT] local dst ids (or -1)
        real = perm >= 0
        gpos_of_node[c * ND + perm[real]] = c * 4 * T2SROWS + pos_of_slot[real]
    sent_pos = np.array([c * 4 * T2SROWS + PT for c in range(NCORES)],
                        dtype=np.int64)

    per_core = _pack_inputs(datas, gpos_of_node, parts, sent_pos)

    # weights: w1e = [W1 | W1@A1s | W1@A1d], h block c-major downstream
    A1s = np.zeros((HID, H1), dtype=np.float32)
    A1d = np.zeros((HID, H1), dtype=np.float32)
    for h in range(H1):
        A1s[h * C1 : (h + 1) * C1, h] = a1s[h]
        A1d[h * C1 : (h + 1) * C1, h] = a1d[h]
    # h block emitted c-major directly: permute W1 columns (and b1)
    cm = (np.arange(HID) % H1) * C1 + (np.arange(HID) // H1)
    w1e = np.concatenate([w1[:, cm], w1 @ A1s, w1 @ A1d], axis=1)   # [128, 80]
    w2cm = w2[cm]
    w2e = np.concatenate([w2cm, w2cm @ a2s.T, w2cm @ a2d.T], axis=1)  # [64, 4]
    b1e = np.tile(b1[cm][None, :], (128, 1)).astype(np.float32)
    b2e = np.tile(b2[None, :], (128, 1)).astype(np.float32)

    # x: pad and permute columns (slab col i*128+p holds node sl*1024+8p+i)
    xp = np.zeros((NPAD, IN), dtype=np.float32)
    xp[:N] = x
    j = np.arange(1024)
    perm1024 = 8 * (j % 128) + j // 128
    permall = (np.arange(NPAD).reshape(-1, 1024) // 1024) * 1024
    permall = (permall + perm1024[None, :]).reshape(-1)
    xpT = np.ascontiguousarray(xp[permall].T.astype(BF16))

    key = (totc, tuple(K.tolist()))
    if key not in _BUILD_CACHE:
        _BUILD_CACHE[key] = _build(K, col0, totc)
    nc = _BUILD_CACHE[key]

    common = dict(x=xpT, w1e=w1e.astype(BF16), w2e=w2e.astype(BF16),
                  b1e=b1e, b2e=b2e)
    in_maps = []
    for c in range(NCORES):
        m = dict(common)
        m.update(per_core[c])
        in_maps.append(m)

    global _LAST_IN_MAPS
    _LAST_IN_MAPS = in_maps
    res = run_bass_kernel_spmd(nc, in_maps, list(range(NCORES)))

    out = np.zeros((N, OUT), dtype=np.float32)
    for c in range(NCORES):
        op = res.results[c]["outp"]       # [PT, 2], row = p*98+t
        perm = datas[c]["perm"]
        real = perm >= 0
        out[c * ND + perm[real]] = op[pos_of_slot[real]]
    return out


# revision 16
# speedup vs baseline: 1.3626x; 1.0489x over previous
"""GAT (2-layer, PyG-style) on 8 Trainium2 NeuronCores via Bass/Tile.

Strategy (dst-sharded, degree-sorted tiles), v2:
- Nodes sharded by dst across 8 cores (12500 each). Per core, dsts are
  degree-sorted and grouped into 98 tiles of 128 (partition = dst).
- Per tile, column 0 gathers the dst's own table row (serving both the
  self-loop edge and al_dst); remaining columns hold in-edges; invalid
  slots gather a zeroed sentinel row so no masking stream is needed.
- Layer tables are 4-node-packed rows (<=32767 rows, int16 dma_gather
  indices); the 4-way select runs as int32-pair predicated copies (half
  the DVE elements of a bf16 select).
- exp(LeakyReLU(a+b)) == max(exp(a)exp(b), exp(a/5)exp(b/5)^...) trick:
  tables store E1=exp(al_src), E2=exp(0.2*al_src); per-tile D1=exp(al_dst),
  D2=exp(0.2*al_dst); per-edge weight w = max(E1*D1, E2*D2). No per-edge
  exp and no LeakyReLU on the hot path.
- h stored c-major so the alpha*h multiply runs in the DVE 2x packed mode
  with the weight broadcast on a middle axis.
- Node phase processes 8 x-tiles per DMA; host pre-permutes x columns so
  each slab writes table1 with a single strided DMA.
- Per-tile accumulators land in a persistent SBUF buffer; softmax
  normalization, ELU and the layer-2 projection run as batched ops after
  the edge phase (no per-tile serial finalize chain).
- Layer-2 per-node features are exchanged via an on-chip AllGather of
  4-packed shards in core-local permuted order.
"""

import numpy as np
import ml_dtypes

BF16 = ml_dtypes.bfloat16

N = 100_000
E = 3_200_000
IN = 128
H1, C1 = 8, 8
HID = H1 * C1          # 64
OUT = 2
NEG = 0.2
NCORES = 8
ND = N // NCORES       # dsts per core: 12500
NT = 98                # tiles per core (98*128 = 12544)
PT = NT * 128          # padded dst slots per core
NPAD = 100_352         # x padded to 98*1024
T1ROWS = NPAD // 4     # 25088 4-pack rows
REC1 = 96              # elems per node record: h(64)|E1(8)|E2(8)|aldst(8)|pad
T1W = 4 * REC1         # 384 bf16 per table1 row (768B)
SENT_ROW = 25_000      # row holding node 100000 (zeroed sentinel)
T2SROWS = PT // 4 + 1  # per-core layer2 shard rows (+1 zero sentinel row)
T2ROWS = T2SROWS * NCORES     # 25096
REC2 = 32              # z0 z1 E1 E2 aldst 0...
T2W = 4 * REC2         # 128 bf16 per table2 row (256B)
PARTC = 16             # columns per tile-part
CPC = 8                # columns per dma_gather call (65 descs; ring holds one
                       # in-flight call per queue -- larger calls wedge)
XS = 8                 # x-tiles per node-phase slab


def _wrap_idx(flat):
    """int16 index array -> [128, n/16] wrapped-in-16-partitions, replicated x8."""
    n = flat.shape[0]
    assert n % 16 == 0
    w = flat.reshape(n // 16, 16).T            # [16, n/16]
    return np.tile(w, (8, 1)).astype(np.int16)  # [128, n/16]


def _plan(src, dst):
    """Host-side index planning. Returns per-core data arrays + common schedule."""
    core = dst // ND
    dloc = dst % ND

    per_core = []
    for c in range(NCORES):
        m = core == c
        s_c = src[m]
        d_c = dloc[m]
        deg = np.bincount(d_c, minlength=ND)  # in-edges, no self loop yet
        order = np.argsort(-deg, kind="stable")  # degree-desc permutation
        perm = np.full(PT, -1, dtype=np.int64)
        perm[:ND] = order
        degp = np.zeros(PT, dtype=np.int64)
        degp[:ND] = deg[order]
        # group in-edges by dst for slot filling
        sort_by_d = np.argsort(d_c, kind="stable")
        s_sorted = s_c[sort_by_d]
        starts = np.zeros(ND + 1, dtype=np.int64)
        np.cumsum(deg, out=starts[1:])
        per_core.append(dict(perm=perm, degp=degp, s_sorted=s_sorted, starts=starts))

    # common K_t schedule: columns per tile = 1 (self/dst col) + max in-degree
    K = np.zeros(NT, dtype=np.int64)
    for t in range(NT):
        mx = 0
        for c in range(NCORES):
            d = per_core[c]["degp"][t * 128 : (t + 1) * 128]
            mx = max(mx, int(d.max()) if d.size else 0)
        K[t] = mx + 1
    col0 = np.zeros(NT, dtype=np.int64)
    pos = 0
    for t in range(NT):
        col0[t] = pos
        pos += K[t]
    ncols = pos

    # per-core slot arrays: node id per (col, partition); -1 = invalid
    datas = []
    for c in range(NCORES):
        pc = per_core[c]
        perm, degp, s_sorted, starts = (
            pc["perm"], pc["degp"], pc["s_sorted"], pc["starts"],
        )
        node1 = np.full((ncols, 128), -1, dtype=np.int64)
        for t in range(NT):
            base = col0[t]
            d_orig = perm[t * 128 : (t + 1) * 128]           # local dst ids, -1 pad
            real = d_orig >= 0
            dg = np.where(real, d_orig, 0)
            node1[base, real] = (c * ND + dg)[real]
            kt = int(K[t])
            if kt > 1:
                st = starts[dg]
                cnt = degp[t * 128 : (t + 1) * 128]
                for j in range(1, kt):
                    sel = (j - 1 < cnt) & real
                    idxs = st + (j - 1)
                    node1[base + j, sel] = s_sorted[np.where(sel, idxs, 0)][sel]
        datas.append(dict(node1=node1, perm=pc["perm"]))
    return datas, K, col0, ncols


def _pack_inputs(datas, gpos_of_node, parts, sent_pos):
    """Build per-core device input arrays from the slot plan."""
    per_core_inputs = []
    for c in range(NCORES):
        node1 = datas[c]["node1"]      # [ncols, 128], -1 invalid
        valid = node1 >= 0
        n1 = np.where(valid, node1, N)              # sentinel node 100000
        idx1_flat = (n1 // 4).astype(np.int16)
        oh1 = np.eye(4, dtype=np.int8)[n1 % 4]      # [ncols, 128, 4]

        g = np.where(valid, gpos_of_node[np.where(valid, node1, 0)],
                     sent_pos[c])
        idx2_flat = (g // 4).astype(np.int16)
        oh2 = np.eye(4, dtype=np.int8)[g % 4]

        w1l, w2l = [], []
        for (c0, nc_) in parts:
            w1l.append(_wrap_idx(idx1_flat[c0 : c0 + nc_].reshape(-1)))
            w2l.append(_wrap_idx(idx2_flat[c0 : c0 + nc_].reshape(-1)))
        per_core_inputs.append(dict(
            idx1=np.concatenate(w1l, axis=1),
            idx2=np.concatenate(w2l, axis=1),
            oh1=np.ascontiguousarray(oh1.transpose(1, 0, 2)),
            oh2=np.ascontiguousarray(oh2.transpose(1, 0, 2)),
        ))
    return per_core_inputs


_BUILD_CACHE = {}


def _build(K, col0, totc):
    import concourse.bass as bass
    import concourse.bacc as bacc
    import concourse.mybir as mybir
    import concourse.tile as tile
    from concourse.masks import make_identity

    f32 = mybir.dt.float32
    bf16 = mybir.dt.bfloat16
    i16 = mybir.dt.int16
    i32 = mybir.dt.int32
    i8 = mybir.dt.int8
    AX = mybir.AxisListType.X
    OP = mybir.AluOpType
    ACT = mybir.ActivationFunctionType

    ncols = int(K.sum())

    nc = bacc.Bacc("TRN2", target_bir_lowering=False, debug=False,
                   num_devices=NCORES, num_swdge_queues=4)

    x = nc.dram_tensor("x", [IN, NPAD], bf16, kind="ExternalInput")
    w1e = nc.dram_tensor("w1e", [IN, 80], bf16, kind="ExternalInput")
    w2e = nc.dram_tensor("w2e", [HID, 4], bf16, kind="ExternalInput")
    idx1 = nc.dram_tensor("idx1", [128, totc], i16, kind="ExternalInput")
    idx2 = nc.dram_tensor("idx2", [128, totc], i16, kind="ExternalInput")
    oh1 = nc.dram_tensor("oh1", [128, ncols, 4], i8, kind="ExternalInput")
    oh2 = nc.dram_tensor("oh2", [128, ncols, 4], i8, kind="ExternalInput")
    b1e = nc.dram_tensor("b1e", [128, HID], f32, kind="ExternalInput")
    b2e = nc.dram_tensor("b2e", [128, OUT], f32, kind="ExternalInput")

    table1 = nc.dram_tensor("table1", [T1ROWS, T1W], bf16, kind="Internal")
    t2shard = nc.dram_tensor("t2shard", [T2SROWS, T2W], bf16, kind="Internal")
    table2 = nc.dram_tensor("table2", [T2ROWS, T2W], bf16, kind="Internal",
                            addr_space="Shared")
    outp = nc.dram_tensor("outp", [PT, OUT], f32, kind="ExternalOutput")

    with tile.TileContext(nc) as tc:
        with (
            tc.tile_pool(name="const", bufs=1) as cpool,
            tc.tile_pool(name="node", bufs=3) as npool,
            tc.tile_pool(name="npsum", bufs=4, space="PSUM") as npsum,
            tc.tile_pool(name="gth", bufs=4) as gpool,
            tc.tile_pool(name="edge", bufs=3) as epool,
            tc.tile_pool(name="esmall", bufs=4) as espool,
            tc.tile_pool(name="slab", bufs=2) as slpool,
            tc.tile_pool(name="accs", bufs=2) as apool,
            tc.tile_pool(name="fin", bufs=2) as fpool,
            tc.tile_pool(name="fpsum", bufs=2, space="PSUM") as fpsum,
        ):
            ident = cpool.tile([128, 128], bf16)
            make_identity(nc, ident[:])
            w1es = cpool.tile([IN, 80], bf16)
            nc.sync.dma_start(out=w1es[:], in_=w1e[:])
            w2es = cpool.tile([HID, 4], bf16)
            nc.sync.dma_start(out=w2es[:], in_=w2e[:])
            b1es = cpool.tile([128, HID], f32)
            nc.sync.dma_start(out=b1es[:], in_=b1e[:])
            b2es = cpool.tile([128, OUT], f32)
            nc.sync.dma_start(out=b2es[:], in_=b2e[:])
            # persistent accumulators + staging
            accTn = cpool.tile([128, NT, HID], bf16)    # num (c-major)
            accTd = cpool.tile([128, NT, H1], f32)      # den
            accT2 = cpool.tile([128, NT, 3], f32)       # num(2)|den(1)
            zball = cpool.tile([128, NT, HID], bf16)    # elu output, c-major
            t2s = cpool.tile([128, NT, REC2], bf16)     # layer2 table staging
            t2raw = cpool.tile([128, NT, 4], f32)       # raw g2 results
            zrow = cpool.tile([128, T1W], bf16)         # sentinel zero row
            nc.vector.memset(zrow[:], 0.0)
            nc.vector.memset(t2s[:], 0.0)

            # ---- node phase: table1 records per node, 8 x-tiles per slab.
            # Host permutes x columns: slab col (i*128+p) holds node
            # sl*1024 + 8p + i, so the table write is a single strided DMA.
            for sl in range(NPAD // 1024):
                eng = nc.sync
                xs = npool.tile([IN, 1024], bf16, tag="xs")
                eng.dma_start(out=xs[:], in_=x[:, sl * 1024 : (sl + 1) * 1024])
                t1s = npool.tile([128, XS, REC1], bf16, tag="t1s")
                for hf in range(2):
                    g1 = npsum.tile([128, 4, 80], f32, tag="g1")
                    for i in range(4):
                        ti = hf * 4 + i
                        nc.tensor.matmul(
                            out=g1[:, i, :],
                            lhsT=xs[:, ti * 128 : (ti + 1) * 128],
                            rhs=w1es[:], start=True, stop=True)
                    sel = t1s[:, hf * 4 : hf * 4 + 4, :]
                    # h (already c-major from the W1 column permutation) + b1
                    nc.vector.tensor_tensor(
                        out=sel[:, :, 0:HID], in0=g1[:, :, 0:HID],
                        in1=b1es[:].unsqueeze(1).to_broadcast([128, 4, HID]),
                        op=OP.add)
                    nc.scalar.activation(out=sel[:, :, 64:72],
                                         in_=g1[:, :, 64:72], func=ACT.Exp)
                    nc.scalar.activation(out=sel[:, :, 72:80],
                                         in_=g1[:, :, 64:72], func=ACT.Exp,
                                         scale=NEG)
                    nc.vector.tensor_copy(out=sel[:, :, 80:88],
                                          in_=g1[:, :, 72:80])
                    nc.vector.memset(sel[:, :, 88:96], 0.0)
                # one strided DMA writes 1024 node records
                dst = table1[:].rearrange("r w -> (r w)").rearrange(
                    "(s p i1 i0 v) -> s p i1 i0 v",
                    p=128, i1=2, i0=4, v=REC1)[sl]
                eng.dma_start(out=dst, in_=t1s[:])
            # zero the sentinel row (node 100000)
            nc.sync.dma_start(out=table1[SENT_ROW : SENT_ROW + 1, :],
                              in_=zrow[0:1, :])

            # ---- edge phase runner
            def edge_phase(layer):
                if layer == 1:
                    idxT, ohT, tabT, EW, RC = idx1, oh1, table1, T1W, REC1
                    NH, NV = H1, HID
                    accn, accd = accTn, accTd
                else:
                    idxT, ohT, tabT, EW, RC = idx2, oh2, table2, T2W, REC2
                    NH, NV = 1, OUT
                    accn = accd = accT2
                RCI = RC // 2           # record in int32 elems
                SEL = (NV + 2 * NH) // 2  # int32 width of contiguous select
                # flat part list; idx/oh prefetched in 16-part slabs
                plist = []
                ioff = 0
                for t in range(NT):
                    kt_full = int(K[t])
                    for p0 in range(0, kt_full, PARTC):
                        kt = min(PARTC, kt_full - p0)
                        plist.append((t, p0, kt, ioff, int(col0[t]) + p0))
                        ioff += kt * 8

                SLAB = 16
                slab_tiles = {}

                def dispatch_slab(k):
                    lo = k * SLAB
                    hi = min(len(plist), lo + SLAB)
                    if lo >= len(plist):
                        return
                    i0 = plist[lo][3]
                    i1 = plist[hi - 1][3] + plist[hi - 1][2] * 8
                    c0 = plist[lo][4]
                    c1 = plist[hi - 1][4] + plist[hi - 1][2]
                    ixs = slpool.tile([128, SLAB * PARTC * 8], i16, tag="ixs")
                    nc.sync.dma_start(out=ixs[:, 0 : i1 - i0],
                                      in_=idxT[:, i0:i1])
                    ohs = slpool.tile([128, SLAB * PARTC, 4], i8, tag="ohs")
                    nc.sync.dma_start(out=ohs[:, 0 : c1 - c0, :],
                                      in_=ohT[:, c0:c1, :])
                    slab_tiles[k] = (ixs, ohs, i0, c0)

                dispatch_slab(0)
                gq = 0
                D1 = D2 = None
                for i, (t, p0, kt, ioff_, col) in enumerate(plist):
                    if True:
                        k = i // SLAB
                        if i % SLAB == 0:
                            dispatch_slab(k + 1)
                            slab_tiles.pop(k - 2, None)
                        first = p0 == 0
                        ixs, ohs, si0, sc0 = slab_tiles[k]
                        io8 = ioff_ - si0
                        oc = col - sc0
                        oh_t = ohs
                        gt = gpool.tile([128, PARTC, EW], bf16,
                                        tag=f"gt{layer}")
                        for cc in range(0, kt, CPC):
                            ncc = min(CPC, kt - cc)
                            nc.gpsimd.dma_gather(
                                gt[:, cc : cc + ncc, :], tabT[:],
                                ixs[:, io8 + cc * 8 : io8 + (cc + ncc) * 8],
                                ncc * 128, ncc * 128, EW, queue_num=gq % 4)
                            gq += 1
                        gti = gt[:, 0:kt, :].bitcast(i32)   # [128, kt, EW/2]
                        V = epool.tile([128, PARTC, NV + 2 * NH], bf16,
                                       tag=f"V{layer}")
                        Vi = V[:, 0:kt, :].bitcast(i32)
                        for s in range(4):
                            nc.vector.copy_predicated(
                                out=Vi,
                                mask=oh_t[:, oc : oc + kt, s : s + 1]
                                    .to_broadcast([128, kt, SEL]),
                                data=gti[:, :, s * RCI : s * RCI + SEL])
                        if first:
                            adt = espool.tile([128, 1, 2 * ((NH + 1) // 2)],
                                             bf16, tag=f"adt{layer}")
                            adti = adt[:].bitcast(i32)
                            na = adti.shape[-1]
                            for s in range(4):
                                nc.vector.copy_predicated(
                                    out=adti,
                                    mask=oh_t[:, oc : oc + 1, s : s + 1]
                                        .to_broadcast([128, 1, na]),
                                    data=gti[:, 0:1,
                                             s * RCI + SEL : s * RCI + SEL + na])
                            D1 = espool.tile([128, 1, NH], bf16,
                                            tag=f"D1{layer}")
                            D2 = espool.tile([128, 1, NH], bf16,
                                            tag=f"D2{layer}")
                            nc.scalar.activation(out=D1[:], in_=adt[:, :, 0:NH],
                                                 func=ACT.Exp)
                            nc.scalar.activation(out=D2[:], in_=adt[:, :, 0:NH],
                                                 func=ACT.Exp, scale=NEG)
                        m1 = espool.tile([128, PARTC, NH], bf16,
                                        tag=f"m1{layer}")
                        m2 = espool.tile([128, PARTC, NH], bf16,
                                        tag=f"m2{layer}")
                        w = espool.tile([128, PARTC, NH], bf16,
                                       tag=f"w{layer}")
                        nc.vector.tensor_tensor(
                            out=m1[:, 0:kt, :], in0=V[:, 0:kt, NV : NV + NH],
                            in1=D1[:].to_broadcast([128, kt, NH]), op=OP.mult)
                        nc.vector.tensor_tensor(
                            out=m2[:, 0:kt, :],
                            in0=V[:, 0:kt, NV + NH : NV + 2 * NH],
                            in1=D2[:].to_broadcast([128, kt, NH]), op=OP.mult)
                        nc.vector.tensor_tensor(
                            out=w[:, 0:kt, :], in0=m1[:, 0:kt, :],
                            in1=m2[:, 0:kt, :], op=OP.max)
                        Wj = epool.tile([128, PARTC, NV], bf16,
                                        tag=f"Wj{layer}")
                        if layer == 1:
                            # c-major V, weight broadcast on middle axis -> 2x
                            nc.vector.tensor_tensor(
                                out=Wj[:, 0:kt, :].rearrange(
                                    "p j (c h) -> p j c h", h=H1),
                                in0=V[:, 0:kt, 0:NV].rearrange(
                                    "p j (c h) -> p j c h", h=H1),
                                in1=w[:, 0:kt, :].unsqueeze(2).to_broadcast(
                                    [128, kt, C1, H1]),
                                op=OP.mult)
                        else:
                            nc.vector.tensor_tensor(
                                out=Wj[:, 0:kt, :], in0=V[:, 0:kt, 0:NV],
                                in1=w[:, 0:kt, :].to_broadcast([128, kt, NV]),
                                op=OP.mult)
                        nslice = (accn[:, t, 0:NV] if layer == 1
                                  else accn[:, t, 0:NV])
                        dslice = (accd[:, t, 0:NH] if layer == 1
                                  else accd[:, t, NV : NV + NH])
                        if first:
                            nc.vector.tensor_reduce(
                                out=nslice,
                                in_=Wj[:, 0:kt, :].rearrange("p j f -> p f j"),
                                axis=AX, op=OP.add)
                            nc.vector.tensor_reduce(
                                out=dslice,
                                in_=w[:, 0:kt, :].rearrange("p j h -> p h j"),
                                axis=AX, op=OP.add)
                        else:
                            rn = apool.tile([128, NV + NH], f32,
                                            tag=f"rn{layer}")
                            nc.vector.tensor_reduce(
                                out=rn[:, 0:NV],
                                in_=Wj[:, 0:kt, :].rearrange("p j f -> p f j"),
                                axis=AX, op=OP.add)
                            nc.vector.tensor_reduce(
                                out=rn[:, NV : NV + NH],
                                in_=w[:, 0:kt, :].rearrange("p j h -> p h j"),
                                axis=AX, op=OP.add)
                            nc.vector.tensor_tensor(
                                out=nslice, in0=nslice, in1=rn[:, 0:NV],
                                op=OP.add)
                            nc.vector.tensor_tensor(
                                out=dslice, in0=dslice,
                                in1=rn[:, NV : NV + NH], op=OP.add)

            with nc.allow_low_precision(reason="bf16 numerator accumulate"):
                edge_phase(1)

            # ---- batched layer-1 finalize: softmax divide + ELU, 14-tile chunks
            CH = 7
            for ch in range(NT // CH):
                den = fpool.tile([128, CH, H1], f32, tag="den")
                nc.vector.tensor_scalar(
                    out=den[:], in0=accTd[:, ch * CH : (ch + 1) * CH, :],
                    scalar1=1e-12, scalar2=None, op0=OP.max)
                rd = fpool.tile([128, CH, H1], f32, tag="rd")
                nc.vector.reciprocal(out=rd[:], in_=den[:])
                z = fpool.tile([128, CH, HID], f32, tag="z")
                nc.vector.tensor_tensor(
                    out=z[:].rearrange("p t (c h) -> p t c h", h=H1),
                    in0=accTn[:, ch * CH : (ch + 1) * CH, :].rearrange(
                        "p t (c h) -> p t c h", h=H1),
                    in1=rd[:].unsqueeze(2).to_broadcast([128, CH, C1, H1]),
                    op=OP.mult)
                # elu
                zp = fpool.tile([128, CH, HID], f32, tag="zp")
                nc.vector.tensor_scalar(out=zp[:], in0=z[:], scalar1=0.0,
                                        scalar2=None, op0=OP.max)
                nc.vector.tensor_scalar(out=z[:], in0=z[:], scalar1=0.0,
                                        scalar2=None, op0=OP.min)
                ez = fpool.tile([128, CH, HID], f32, tag="ez")
                nc.scalar.activation(out=ez[:], in_=z[:], func=ACT.Exp)
                nc.vector.tensor_scalar(out=ez[:], in0=ez[:], scalar1=-1.0,
                                        scalar2=None, op0=OP.add)
                nc.vector.tensor_tensor(
                    out=zball[:, ch * CH : (ch + 1) * CH, :], in0=zp[:],
                    in1=ez[:], op=OP.add)

            # ---- layer-2 projection per tile + staging
            for t in range(NT):
                zT_ps = fpsum.tile([HID, 128], bf16, tag="zTp")
                nc.tensor.transpose(out=zT_ps[:], in_=zball[:, t, :],
                                    identity=ident[:])
                zTs = fpool.tile([HID, 128], bf16, tag="zTs")
                nc.vector.tensor_copy(out=zTs[:], in_=zT_ps[:])
                g2 = fpsum.tile([128, 4], f32, tag="g2p")
                nc.tensor.matmul(out=g2[:], lhsT=zTs[:], rhs=w2es[:],
                                 start=True, stop=True)
                nc.vector.tensor_copy(out=t2raw[:, t, :], in_=g2[:])
            # batched staging ops
            nc.vector.tensor_tensor(
                out=t2s[:, :, 0:OUT], in0=t2raw[:, :, 0:OUT],
                in1=b2es[:].unsqueeze(1).to_broadcast([128, NT, OUT]),
                op=OP.add)
            nc.scalar.activation(out=t2s[:, :, 2:3], in_=t2raw[:, :, 2:3],
                                 func=ACT.Exp)
            nc.scalar.activation(out=t2s[:, :, 3:4], in_=t2raw[:, :, 2:3],
                                 func=ACT.Exp, scale=NEG)
            nc.vector.tensor_copy(out=t2s[:, :, 4:5], in_=t2raw[:, :, 3:4])
            # single strided DMA: slot (p, t) -> position p*98+t
            t2dst = t2shard[0 : PT // 4, :].rearrange("r w -> (r w)").rearrange(
                "(p t v) -> p t v", p=128, t=NT)
            nc.sync.dma_start(out=t2dst, in_=t2s[:])
            # zero sentinel row (gathered by invalid layer-2 slots)
            nc.sync.dma_start(out=t2shard[PT // 4 : T2SROWS, :],
                              in_=zrow[0:1, 0:T2W])

            # ---- exchange layer-2 node features
            nc.gpsimd.collective_compute(
                "AllGather",
                OP.bypass,
                replica_groups=[list(range(NCORES))],
                ins=[t2shard[:]],
                outs=[table2[:]],
            )

            with nc.allow_low_precision(reason="bf16 numerator accumulate"):
                edge_phase(2)

            # ---- batched layer-2 finalize
            den2 = fpool.tile([128, NT, 1], f32, tag="den2")
            nc.vector.tensor_scalar(out=den2[:], in0=accT2[:, :, 2:3],
                                    scalar1=1e-12, scalar2=None, op0=OP.max)
            rd2 = fpool.tile([128, NT, 1], f32, tag="rd2")
            nc.vector.reciprocal(out=rd2[:], in_=den2[:])
            o2 = fpool.tile([128, NT, OUT], f32, tag="o2")
            nc.vector.tensor_tensor(
                out=o2[:], in0=accT2[:, :, 0:OUT],
                in1=rd2[:].to_broadcast([128, NT, OUT]), op=OP.mult)
            odst = outp[:].rearrange("r w -> (r w)").rearrange(
                "(p t v) -> p t v", p=128, t=NT)
            nc.sync.dma_start(out=odst, in_=o2[:])

    nc.compile()
    return nc


def kernel(**inputs):
    from concourse.bass_utils import run_bass_kernel_spmd

    x = np.asarray(inputs["x"], dtype=np.float32)
    ei = np.asarray(inputs["edge_index"]).astype(np.int64)
    w1 = np.asarray(inputs["W1"], dtype=np.float32)
    a1s = np.asarray(inputs["a1_src"], dtype=np.float32)
    a1d = np.asarray(inputs["a1_dst"], dtype=np.float32)
    b1 = np.asarray(inputs["b1"], dtype=np.float32)
    w2 = np.asarray(inputs["W2"], dtype=np.float32)
    a2s = np.asarray(inputs["a2_src"], dtype=np.float32)
    a2d = np.asarray(inputs["a2_dst"], dtype=np.float32)
    b2 = np.asarray(inputs["b2"], dtype=np.float32)

    src = ei[0]
    dst = ei[1]

    datas, K, col0, ncols = _plan(src, dst)
    parts = []
    for t in range(NT):
        for c in range(0, int(K[t]), PARTC):
            parts.append((int(col0[t]) + c, min(PARTC, int(K[t]) - c)))
    totc = sum(nc_ * 8 for _, nc_ in parts)

    # global position of each node for the L2 table: pos = c*4*T2SROWS +
    # p*98 + t where the node is dst slot (t, p) on core c.
    gpos_of_node = np.zeros(N, dtype=np.int64)
    s_old = np.arange(PT)
    tt = s_old // 128
    pp = s_old % 128
    pos_of_slot = pp * NT + tt
    for c in range(NCORES):
        perm = datas[c]["perm"]  # [PT] local dst ids (or -1)
        real = perm >= 0
        gpos_of_node[c * ND + perm[real]] = c * 4 * T2SROWS + pos_of_slot[real]
    sent_pos = np.array([c * 4 * T2SROWS + PT for c in range(NCORES)],
                        dtype=np.int64)

    per_core = _pack_inputs(datas, gpos_of_node, parts, sent_pos)

    # weights: w1e = [W1 | W1@A1s | W1@A1d], h block c-major downstream
    A1s = np.zeros((HID, H1), dtype=np.float32)
    A1d = np.zeros((HID, H1), dtype=np.float32)
    for h in range(H1):
        A1s[h * C1 : (h + 1) * C1, h] = a1s[h]
        A1d[h * C1 : (h + 1) * C1, h] = a1d[h]
    # h block emitted c-major directly: permute W1 columns (and b1)
    cm = (np.arange(HID) % H1) * C1 + (np.arange(HID) // H1)
    w1e = np.concatenate([w1[:, cm], w1 @ A1s, w1 @ A1d], axis=1)   # [128, 80]
    w2cm = w2[cm]
    w2e = np.concatenate([w2cm, w2cm @ a2s.T, w2cm @ a2d.T], axis=1)  # [64, 4]
    b1e = np.tile(b1[cm][None, :], (128, 1)).astype(np.float32)
    b2e = np.tile(b2[None, :], (128, 1)).astype(np.float32)

    # x: pad and permute columns (slab col i*128+p holds node sl*1024+8p+i)
    xp = np.zeros((NPAD, IN), dtype=np.float32)
    xp[:N] = x
    j = np.arange(1024)
    perm1024 = 8 * (j % 128) + j // 128
    permall = (np.arange(NPAD).reshape(-1, 1024) // 1024) * 1024
    permall = (permall + perm1024[None, :]).reshape(-1)
    xpT = np.ascontiguousarray(xp[permall].T.astype(BF16))

    key = (totc, tuple(K.tolist()))
    if key not in _BUILD_CACHE:
        _BUILD_CACHE[key] = _build(K, col0, totc)
    nc = _BUILD_CACHE[key]

    common = dict(x=xpT, w1e=w1e.astype(BF16), w2e=w2e.astype(BF16),
                  b1e=b1e, b2e=b2e)
    in_maps = []
    for c in range(NCORES):
        m = dict(common)
        m.update(per_core[c])
        in_maps.append(m)

    global _LAST_IN_MAPS
    _LAST_IN_MAPS = in_maps
    res = run_bass_kernel_spmd(nc, in_maps, list(range(NCORES)))

    out = np.zeros((N, OUT), dtype=np.float32)
    for c in range(NCORES):
        op = res.results[c]["outp"]       # [PT, 2], row = p*98+t
        perm = datas[c]["perm"]
        real = perm >= 0
        out[c * ND + perm[real]] = op[pos_of_slot[real]]
    return out
